# revision 65
# baseline (speedup 1.0000x reference)
"""MultiHeadAttention TRN2 Bass kernel.

Problem: B=4, S=2048, D=768, H=12 heads (DK=64).
Sharding: 8 cores = (batch b in 0..3) x (head-half in 0..1); each core
computes 6 heads of one batch element end-to-end (tensor-parallel over
heads within a batch). Host pre-transposes activations to [D, S] (and
casts to bf16 in the default fast path), slices projection weights per
head-half, and sums the two partial outputs per batch (+ bv@Wo + bo
correction, exact because softmax rows sum to 1).

Key optimization vs the dense formulation: the mask is per-key (same
for every query/head in a batch), so masked keys are removed ENTIRELY
on the host -- k/v are gathered down to the ~50% kept keys and padded
with zeros to SKV (multiple of 128, >= 1024). mv[s]=1 marks real keys,
0 marks padding; it is folded into vh_aug so padded keys contribute
exactly 0 to both the softmax numerator and denominator. This cuts
k/v-proj, scores, exp, and attn@V work by ~44% with bit-identical
semantics to the -inf mask.

On-core math (SKV = padded kept-key count, NKT = SKV/128):
  qh^T[384, S]: lhsT=Wq tile [Din,dout], rhs=q^T tile [Din,s] (+bq)
  kh^T[384, SKV] likewise; vh natural [SKV, 390] via lhsT=v^T, rhs=Wv:
    vh_aug[s, 65j..65j+64] = [mv(s)*vh_head_j(s, :), mv(s)]
  S^T[k, q] = kh_head^T.T @ qh_head^T  (contraction d=64; the two heads
    of a pair land on disjoint PE row quadrants -> they co-execute, and
    share one [128,1024] PSUM tile -> one exp per step)
  P^T = exp(S^T * 0.125)               (ACT, fused scale, no max-sub)
  ctx_aug^T[0:65, q] += vh_aug_j[kc].T @ P^T[kc]  over NKT k-chunks
    rows 0..63 = unnormalized ctx^T, row 64 = softmax denominator
  rs = reciprocal_approx_fast(denom copied to SBUF); bcast on gpsimd;
  cn = ctx^T * rs   (drains deferred so the PE pipeline never waits)
  out[q, 768] = sum_dt cn[dt].T @ Wo tiles  (per 128-q chunk), bf16 out

Scheduling notes (why it runs at ~194us instead of the 394us of the
dense ancestor):
- masked-key compaction cuts scores/exp/attn@V/kv-proj by ~44%
- phase 2 is co-bound: PE ~753ns/step + fillers vs ACT exp ~1.1us/step;
  deferred q-proj and O-proj run as fillers in fixed kc slots, placed
  >=4 steps after the drain that produces their input (in-order PE: a
  stalled filler blocks everything behind it)
- every dma_start rides ONE ~20GB/s hardware ring: big loads are split
  into ~100-150KB pieces issued round-robin over the sync/gpsimd/scalar
  queue sequencers (~600ns issue cost each), in consumption order
- the PE ramps 0.65->1.2->2.4GHz over ~3us of continuous work and any
  idle gap resets it: keep-warm dummy matmuls bridge the final drain
  wait so the tail O-projs run at full clock
- the first two scores+exp are hoisted before v-proj so the scalar
  engine's 108-exp stream starts as early as possible
"""

import os
import sys
import types
from contextlib import ExitStack

import ml_dtypes
import numpy as np

import concourse.bacc as bacc
import concourse.bass as bass
import concourse.mybir as mybir
import concourse.tile as tile
from concourse import bass_utils
from concourse.bass import ts, ds

F32 = mybir.dt.float32
F32R = mybir.dt.float32r
BF16 = mybir.dt.bfloat16

D = 768        # model dim
DH = 384       # per-core head dim (6 heads x 64)
HPC = 6        # heads per core
VW = HPC * 65  # vh_aug free width (390)


def build_nc(S=2048, SKV=1152, bf16=True):
    nc = bacc.Bacc("TRN2", target_bir_lowering=False, debug=False)

    MMD = BF16 if bf16 else F32R    # matmul operand dtype
    NKT = SKV // 128                # 128-wide k-tiles
    assert SKV % 128 == 0 and NKT >= 8
    QBW = min(512, S)               # attention q-block width
    NQB = S // QBW                  # q blocks
    CWQ = min(1024, S)              # q-proj s-chunk width
    # k-proj free-dim chunk: largest 128*d <= 512 with d | NKT
    CWK = next(128 * d for d in (4, 3, 2, 1) if NKT % d == 0)

    qT = nc.dram_tensor("qT", [D, S], MMD, kind="ExternalInput").ap()
    kT = nc.dram_tensor("kT", [D, SKV], MMD, kind="ExternalInput").ap()
    vT = nc.dram_tensor("vT", [D, SKV], MMD, kind="ExternalInput").ap()
    wq = nc.dram_tensor("wq", [D, DH], MMD, kind="ExternalInput").ap()
    wk = nc.dram_tensor("wk", [D, DH], MMD, kind="ExternalInput").ap()
    wv = nc.dram_tensor("wv", [D, DH], MMD, kind="ExternalInput").ap()
    wo = nc.dram_tensor("wo", [DH, D], MMD, kind="ExternalInput").ap()
    # col 0..2 = bq (3 dt-tiles), 3..5 = bk, 6..6+NKT = mv (padding flag)
    smalls = nc.dram_tensor("smalls", [128, 6 + NKT], F32, kind="ExternalInput").ap()
    out = nc.dram_tensor("out", [S, D], BF16, kind="ExternalOutput").ap()

    with tile.TileContext(nc) as tc, ExitStack() as ctx:
        P = 128
        wpool = ctx.enter_context(tc.tile_pool(name="w", bufs=1))
        xin = ctx.enter_context(tc.tile_pool(name="xin", bufs=12))
        persist = ctx.enter_context(tc.tile_pool(name="persist", bufs=1))
        ppool = ctx.enter_context(tc.tile_pool(name="p", bufs=3))
        small = ctx.enter_context(tc.tile_pool(name="small", bufs=2))
        outp = ctx.enter_context(tc.tile_pool(name="outp", bufs=2))
        psA = ctx.enter_context(tc.tile_pool(name="psA", bufs=2, space="PSUM"))
        psB = ctx.enter_context(tc.tile_pool(name="psB", bufs=4, space="PSUM"))

        # Round-robin DMA issue across 4 engine sequencers: each dma_start
        # costs ~600ns of issue time on its engine, so spreading the ~45
        # phase-1 loads over 4 queues (in dependency order: wk+kt first)
        # cuts the serial descriptor-issue head from ~20us to ~4us.
        # The scalar (ACT) queue must drain before exp(0) can dispatch --
        # every DMA issued on it costs ~600ns of sequencer time ahead of the
        # exp stream. So scalar only gets the earliest wave (wk/kt), and is
        # dropped from the rotation afterwards.
        dmaq = [nc.sync, nc.gpsimd, nc.scalar]
        dqi = [0]

        def dq_start(dst, src):
            dmaq[dqi[0] % len(dmaq)].dma_start(dst, src)
            dqi[0] += 1

        # ---- constants / small tensors ----
        wq_sb = [wpool.tile([P, DH], MMD, name=f"wq{c}", tag=f"wq{c}") for c in range(6)]
        wk_sb = [wpool.tile([P, DH], MMD, name=f"wk{c}", tag=f"wk{c}") for c in range(6)]
        wv_sb = [wpool.tile([P, DH], MMD, name=f"wv{c}", tag=f"wv{c}") for c in range(6)]
        wo_sb = [wpool.tile([P, D], MMD, name=f"wo{c}", tag=f"wo{c}") for c in range(3)]
        sm_sb = wpool.tile([128, 6 + NKT], F32, tag="smalls")
        # DMA priority order = compute order: q-proj runs first (so its data
        # loads first), k-proj next (kt fully landed by then -> no mid-kproj
        # DMA stalls that would reset the PE p-state), v/o/deferred-q last.
        # Each dma_start rides a single ~20GB/s hardware ring, so big loads
        # are split into ~128-150KB pieces to spread across the 16 rings,
        # issued in the order compute consumes them (kproj, vproj, qproj).
        # kt/vt pieces are split on the column boundaries the consuming
        # matmul chains use, and issued chunk-0-of-every-tile first, so the
        # first kproj/vproj chains start as early as possible.
        kt = [xin.tile([P, SKV], MMD, name="xin", tag="xin") for c in range(6)]
        for c in range(6):
            dq_start(wk_sb[c][:], wk[ts(c, P), :])
        for sc in range(SKV // CWK):
            for c in range(6):
                dq_start(kt[c][:, ts(sc, CWK)], kT[ts(c, P), ts(sc, CWK)])
        dmaq.pop()  # scalar queue must be free well before the first exp
        dq_start(sm_sb[:], smalls[:, :])
        bq_sb = [sm_sb[:, t : t + 1] for t in range(3)]
        bk_sb = [sm_sb[:, 3 + t : 4 + t] for t in range(3)]
        mv_sb = [sm_sb[:, 6 + st : 7 + st] for st in range(NKT)]
        ones6 = wpool.tile([P, HPC], F32, tag="ones6")
        nc.vector.memset(ones6[:], 1.0)
        qt0 = [xin.tile([P, CWQ], MMD, name="xin", tag="xin") for c in range(6)]
        for c in range(6):
            dq_start(wq_sb[c][:], wq[ts(c, P), :])
            dq_start(qt0[c][:, 0:512], qT[ts(c, P), 0:512])
        vt = [xin.tile([P, SKV], MMD, name="xin", tag="xin") for c in range(6)]
        for c in range(6):
            dq_start(wv_sb[c][:], wv[ts(c, P), :])
        for sc in range(SKV // CWK):
            for c in range(6):
                dq_start(vt[c][:, ts(sc, CWK)], vT[ts(c, P), ts(sc, CWK)])
        for c in range(6):
            dq_start(qt0[c][:, 512:CWQ], qT[ts(c, P), 512:CWQ])
        for c in range(3):
            dq_start(wo_sb[c][:], wo[ts(c, P), :])
        qproj_xt = {0: qt0}
        for sc in range(1, S // CWQ):
            qproj_xt[sc] = [
                xin.tile([P, CWQ], MMD, name="xin", tag="xin") for c in range(6)
            ]
            for c in range(6):
                dq_start(qproj_xt[sc][c][:, 0:512], qT[ts(c, P), ds(sc * CWQ, 512)])
                dq_start(
                    qproj_xt[sc][c][:, 512:CWQ],
                    qT[ts(c, P), ds(sc * CWQ + 512, 512)],
                )

        # ---- persistent activations ----
        khT = [persist.tile([P, SKV], MMD, name=f"khT{t}", tag=f"khT{t}") for t in range(3)]
        qhT = [persist.tile([P, S], MMD, name=f"qhT{t}", tag=f"qhT{t}") for t in range(3)]
        vh = [persist.tile([P, VW], MMD, name=f"vh{st}", tag=f"vh{st}") for st in range(NKT)]
        cn = [persist.tile([P, S], MMD, name=f"cn{t}", tag=f"cn{t}") for t in range(3)]

        # ---- phase 1: k-proj, v-proj, then q-proj of the first 512 cols
        # (matching DMA arrival order); the rest of q-proj is phase-2 filler.
        # Keep-warm matmuls on the already-arrived first kt chunk bridge the
        # DMA wait before the last k-chunk so kproj's tail (and everything
        # after) runs at full PE clock instead of mid p-state.
        wrm = psB.tile([P, 512], F32, name="psB", tag="psB")
        NSC = SKV // CWK
        for sc in range(NSC):
            for dt in range(3):
                ps = psA.tile([P, CWK], F32, name="psA", tag="psA")
                for c in range(6):
                    nc.tensor.matmul(
                        ps[:],
                        lhsT=wk_sb[c][:, ts(dt, P)],
                        rhs=kt[c][:, ts(sc, CWK)],
                        start=(c == 0),
                        stop=(c == 5),
                    )
                nc.vector.tensor_scalar_add(
                    out=khT[dt][:, ts(sc, CWK)], in0=ps[:],
                    scalar1=bk_sb[dt],
                )
            if sc == NSC - 2:
                for _ in range(16):
                    nc.tensor.matmul(
                        wrm[:, 0:CWK], lhsT=kt[0][:, 0:128],
                        rhs=kt[0][:, ts(0, CWK)], start=True, stop=True,
                    )

        def vproj_sub(st):
            ps = psB.tile([P, 512], F32, name="psB", tag="psB")
            for c in range(6):
                nc.tensor.matmul(
                    ps[:, :DH],
                    lhsT=vt[c][:, ts(st, P)],
                    rhs=wv_sb[c][:],
                    start=(c == 0),
                    stop=(c == 5),
                )
            vh3 = vh[st].rearrange("p (h c) -> p h c", c=65)
            nc.vector.tensor_scalar_mul(
                out=vh3[:, :, 0:64],
                in0=ps[:, :DH].rearrange("p (h c) -> p h c", c=64),
                scalar1=mv_sb[st],
            )
            nc.vector.tensor_scalar_mul(
                out=vh3[:, :, 64:65],
                in0=ones6[:].rearrange("p (h c) -> p h c", c=1),
                scalar1=mv_sb[st],
            )

        for dt in range(3):
            ps = psA.tile([P, 512], F32, name="psA", tag="psA")
            for c in range(6):
                nc.tensor.matmul(
                    ps[:],
                    lhsT=wq_sb[c][:, ts(dt, P)],
                    rhs=qt0[c][:, ts(0, 512)],
                    start=(c == 0),
                    stop=(c == 5),
                )
            nc.vector.tensor_scalar_add(
                out=qhT[dt][:, ts(0, 512)], in0=ps[:],
                scalar1=bq_sb[dt],
            )
        pend_qproj = [(0, dt, 1) for dt in range(3)] + [
            (sc, dt, u)
            for sc in range(1, S // CWQ)
            for dt in range(3)
            for u in range(CWQ // 512)
        ]

        # ---- phase 2: attention, head-pair steps ----
        # Each step handles BOTH heads of a pair for one k-chunk: the two
        # scores matmuls live in disjoint PE row groups (base partition 0
        # and 64) and share one [128,1024] PSUM tile (head A in cols 0:512,
        # head B in 512:1024) -> one exp per step. Scores run 2 steps ahead
        # of attn@V. Drains and filler work (deferred q-proj, O-proj) go to
        # psB spare slots, never stalling the scores pipeline on psA.
        hq = [(pr, qb) for qb in range(NQB) for pr in range(3)]
        steps = [(pr, qb, kc) for (pr, qb) in hq for kc in range(NKT)]

        ctx_ps = {}
        st_ps = {}

        def scores(pr, qb, kc):
            ps = psA.tile([P, 1024], F32, name="psA", tag="psA")
            for hh in range(2):
                nc.tensor.matmul(
                    ps[:, ts(hh, 512)],
                    lhsT=khT[pr][64 * hh : 64 * hh + 64, ts(kc, P)],
                    rhs=qhT[pr][64 * hh : 64 * hh + 64, ts(qb, QBW)],
                    start=True,
                    stop=True,
                )
            st_ps[(pr, qb, kc)] = ps

        # Hoist the first DEPTH scores ahead of v-proj: they only need
        # khT/qhT, so the scalar engine starts the exp stream while the PE
        # is still working through v-proj.
        scores(*steps[0])
        scores(*steps[1])
        for st in range(NKT - 3):
            vproj_sub(st)

        def attnv(pr, qb, kc, pt):
            for hh in range(2):
                h = 2 * pr + hh
                nc.tensor.matmul(
                    ctx_ps[(h, qb)][0:65, :],
                    lhsT=vh[kc][:, ds(65 * h, 65)],
                    rhs=pt[:, ts(hh, 512)],
                    start=(kc == 0),
                    stop=(kc == NKT - 1),
                )

        def drain(h, qb):
            """Normalize + store ctx for a finished (h, qb)."""
            dt, pb = h // 2, 64 * (h % 2)
            cps = ctx_ps.pop((h, qb))
            den = small.tile([1, QBW], F32, name="den", tag="den")
            nc.vector.tensor_copy(den[:], cps[64:65, :])
            rs = small.tile([1, QBW], F32, name="rs", tag="rs")
            nc.vector.reciprocal_approx_fast(rs[:], den[:])
            bcs = small.tile([64, QBW], F32, name="bcs", tag="bcs")
            nc.gpsimd.partition_broadcast(bcs[:], rs[:])
            if pb == 0:
                nc.vector.tensor_tensor(
                    out=cn[dt][0:64, ts(qb, QBW)],
                    in0=cps[0:64, :],
                    in1=bcs[:],
                    op=mybir.AluOpType.mult,
                )
            else:
                tmp = small.tile([64, QBW], MMD, name="tmp", tag="tmp")
                nc.vector.tensor_tensor(
                    out=tmp[:], in0=cps[0:64, :], in1=bcs[:],
                    op=mybir.AluOpType.mult,
                )
                nc.gpsimd.dma_start(cn[dt][64:128, ts(qb, QBW)], tmp[:])

        oq = [nc.sync, nc.gpsimd]

        # Fillers (deferred q-proj, O-proj) are split into <=3-matmul micro
        # pieces chained via open PSUM accumulation, dispatched ONE per step:
        # a chunky filler between two scores delays the next exp (the phase-2
        # pacer) by its full length, costing ~2us per occurrence. Entries are
        # (allocates_psB, chain_len, fn); allocations only happen at kc >= 4
        # (after the previous group's ctx tiles are released) and early
        # enough that the chain closes within its group.
        pend_fill = []

        def queue_qproj(sc, dt, u):
            box = {}

            def p1():
                box["ps"] = psB.tile([P, 512], F32, name="psB", tag="psB")
                for c in range(3):
                    nc.tensor.matmul(
                        box["ps"][:],
                        lhsT=wq_sb[c][:, ts(dt, P)],
                        rhs=qproj_xt[sc][c][:, ts(u, 512)],
                        start=(c == 0),
                        stop=False,
                    )

            def p2():
                for c in range(3, 6):
                    nc.tensor.matmul(
                        box["ps"][:],
                        lhsT=wq_sb[c][:, ts(dt, P)],
                        rhs=qproj_xt[sc][c][:, ts(u, 512)],
                        start=False,
                        stop=(c == 5),
                    )
                nc.vector.tensor_scalar_add(
                    out=qhT[dt][:, ds(sc * CWQ + u * 512, 512)],
                    in0=box["ps"][:], scalar1=bq_sb[dt],
                )

            pend_fill.append((True, 2, p1))
            pend_fill.append((False, 0, p2))

        def queue_oproj(qc):
            box = {}

            def a1():
                box["ups"] = psB.tile([P, 512], F32, name="psB", tag="psB")
                for dt in range(2):
                    nc.tensor.matmul(
                        box["ups"][:],
                        lhsT=cn[dt][:, ts(qc, P)],
                        rhs=wo_sb[dt][:, ds(0, 512)],
                        start=(dt == 0),
                        stop=False,
                    )

            def a2():
                nc.tensor.matmul(
                    box["ups"][:],
                    lhsT=cn[2][:, ts(qc, P)],
                    rhs=wo_sb[2][:, ds(0, 512)],
                    start=False,
                    stop=True,
                )
                box["ot"] = outp.tile([P, D], MMD, name="ot", tag="ot")
                nc.vector.tensor_copy(box["ot"][:, 0:512], box["ups"][:, 0:512])
                oq[qc % len(oq)].dma_start(
                    out[ts(qc, P), 0:512], box["ot"][:, 0:512]
                )

            def b1():
                box["ups2"] = psB.tile([P, 256], F32, name="psB2", tag="psB")
                for dt in range(2):
                    nc.tensor.matmul(
                        box["ups2"][:, 0:256],
                        lhsT=cn[dt][:, ts(qc, P)],
                        rhs=wo_sb[dt][:, ds(512, 256)],
                        start=(dt == 0),
                        stop=False,
                    )

            def b2():
                nc.tensor.matmul(
                    box["ups2"][:, 0:256],
                    lhsT=cn[2][:, ts(qc, P)],
                    rhs=wo_sb[2][:, ds(512, 256)],
                    start=False,
                    stop=True,
                )
                nc.vector.tensor_copy(box["ot"][:, 512:768], box["ups2"][:, 0:256])
                oq[(qc + 1) % len(oq)].dma_start(
                    out[ts(qc, P), 512:768], box["ot"][:, 512:768]
                )

            pend_fill.append((True, 2, a1))
            pend_fill.append((False, 0, a2))
            pend_fill.append((True, 2, b1))
            pend_fill.append((False, 0, b2))

        def queue_vproj(st):
            box = {}

            def v1():
                box["ps"] = psB.tile([P, 512], F32, name="psB", tag="psB")
                for c in range(3):
                    nc.tensor.matmul(
                        box["ps"][:, :DH],
                        lhsT=vt[c][:, ts(st, P)],
                        rhs=wv_sb[c][:],
                        start=(c == 0),
                        stop=False,
                    )

            def v2():
                for c in range(3, 6):
                    nc.tensor.matmul(
                        box["ps"][:, :DH],
                        lhsT=vt[c][:, ts(st, P)],
                        rhs=wv_sb[c][:],
                        start=False,
                        stop=(c == 5),
                    )
                vh3 = vh[st].rearrange("p (h c) -> p h c", c=65)
                nc.vector.tensor_scalar_mul(
                    out=vh3[:, :, 0:64],
                    in0=box["ps"][:, :DH].rearrange("p (h c) -> p h c", c=64),
                    scalar1=mv_sb[st],
                )
                nc.vector.tensor_scalar_mul(
                    out=vh3[:, :, 64:65],
                    in0=ones6[:].rearrange("p (h c) -> p h c", c=1),
                    scalar1=mv_sb[st],
                )

            pend_fill.append((True, 2, v1))
            pend_fill.append((False, 0, v2))

        # last 3 v-proj chunks ride group-0 fill slots (no old ctx tiles
        # there, so psB allocation is safe at any kc); 1 piece/step keeps
        # the exp stream fed
        for st in range(NKT - 3, NKT):
            queue_vproj(st)
        for e in pend_qproj:
            queue_qproj(*e)

        DEPTH = 2
        pend_drain = []
        for n, (pr, qb, kc) in enumerate(steps):
            if kc == 0:
                for hh in range(2):
                    ctx_ps[(2 * pr + hh, qb)] = psB.tile(
                        [P, QBW], F32, name="psB", tag="psB"
                    )[0:65, :]
            pt = ppool.tile([P, 1024], MMD, name="pt", tag="pt")
            nc.scalar.activation(
                pt[:], st_ps.pop((pr, qb, kc))[:],
                mybir.ActivationFunctionType.Exp, scale=0.125,
            )
            if n + DEPTH < len(steps):
                scores(*steps[n + DEPTH])
            attnv(pr, qb, kc, pt)
            if kc in (0, 1) and pend_drain:
                hd, qd = pend_drain.pop(0)
                drain(hd, qd)
                if hd == HPC - 1:
                    for qcx in range(qd * (QBW // P), (qd + 1) * (QBW // P)):
                        queue_oproj(qcx)
            elif pend_fill:
                na, cl, fn = pend_fill[0]
                if (not na) or (kc <= NKT - cl and (kc >= 4 or n < NKT)):
                    pend_fill.pop(0)
                    fn()
            if kc == NKT - 1:
                pend_drain.extend([(2 * pr, qb), (2 * pr + 1, qb)])
        while pend_fill:
            pend_fill.pop(0)[2]()
        # tail: keep-warm matmuls bridge the drain-wait gap so the final
        # o-proj matmuls run at full PE clock instead of mid p-state
        wps = psA.tile([P, 512], F32, name="psA", tag="psA")
        for _ in range(26):
            nc.tensor.matmul(
                wps[:], lhsT=khT[0][:, 0:128], rhs=khT[0][:, 0:512],
                start=True, stop=True,
            )
        # batch the final drains phase-by-phase so the two DVE chains
        # and the two gpsimd broadcasts interleave instead of serializing
        infos = []
        for hd, qd in pend_drain:
            dt, pb = hd // 2, 64 * (hd % 2)
            cps = ctx_ps.pop((hd, qd))
            den = small.tile([1, QBW], F32, name="den", tag="den")
            nc.vector.tensor_copy(den[:], cps[64:65, :])
            rs = small.tile([1, QBW], F32, name="rs", tag="rs")
            nc.vector.reciprocal_approx_fast(rs[:], den[:])
            bcs = small.tile([64, QBW], F32, name="bcs", tag="bcs")
            nc.gpsimd.partition_broadcast(bcs[:], rs[:])
            infos.append((hd, qd, dt, pb, cps, bcs))
        for hd, qd, dt, pb, cps, bcs in infos:
            if pb == 0:
                nc.vector.tensor_tensor(
                    out=cn[dt][0:64, ts(qd, QBW)],
                    in0=cps[0:64, :], in1=bcs[:],
                    op=mybir.AluOpType.mult,
                )
            else:
                tmp = small.tile([64, QBW], MMD, name="tmp", tag="tmp")
                nc.vector.tensor_tensor(
                    out=tmp[:], in0=cps[0:64, :], in1=bcs[:],
                    op=mybir.AluOpType.mult,
                )
                nc.gpsimd.dma_start(cn[dt][64:128, ts(qd, QBW)], tmp[:])
            if hd == HPC - 1:
                for qcx in range(qd * (QBW // P), (qd + 1) * (QBW // P)):
                    queue_oproj(qcx)
        oq.append(nc.scalar)  # scalar queue is free once the exps are done
        while pend_fill:
            pend_fill.pop(0)[2]()

    nc.compile()
    return nc


_NC_CACHE = {}


def _get_nc(S, SKV, bf16=True):
    key = (S, SKV, bf16)
    if key not in _NC_CACHE:
        _NC_CACHE[key] = build_nc(S, SKV, bf16)
    return _NC_CACHE[key]


def _install_ntff_hook():
    try:
        mod = types.ModuleType("antenv.axon_hooks")
        state = {"hook": None}
        mod.set_axon_ntff_profile_hook = lambda h: state.__setitem__("hook", h)
        mod.get_axon_ntff_profile_hook = lambda: state["hook"]
        sys.modules["antenv.axon_hooks"] = mod
        from trn_agent_boot.trn_boot import _ntff_profile_via_ctypes

        mod.set_axon_ntff_profile_hook(
            _ntff_profile_via_ctypes("/opt/axon/libaxon_pjrt.so")
        )
        bass_utils.upload_artifacts = lambda tmpdir: "local://" + tmpdir
        return state["hook"] is not None
    except Exception:
        return False


def run_cores(in_maps, S=2048, SKV=1152, bf16=True, profile=False):
    nc = _get_nc(S, SKV, bf16)
    trace = bool(profile) and _install_ntff_hook()
    res = bass_utils.run_bass_kernel_spmd(
        nc, in_maps, core_ids=list(range(len(in_maps))), trace=trace
    )
    return res


def make_in_maps(q, k, v, mask, Wq, bq, Wk, bk, Wv, Wo, bf16=True):
    B, S, _ = q.shape
    mmd = ml_dtypes.bfloat16 if bf16 else np.float32
    q = np.asarray(q, np.float32)
    k = np.asarray(k, np.float32)
    v = np.asarray(v, np.float32)
    keep = ~np.asarray(mask).reshape(B, S)
    counts = keep.sum(axis=1)
    SKV = max(1024, int(-(-int(counts.max()) // 128)) * 128)
    Wq, Wk, Wv, Wo = (np.asarray(a, np.float32) for a in (Wq, Wk, Wv, Wo))
    bq, bk = np.asarray(bq, np.float32), np.asarray(bk, np.float32)
    in_maps = []
    NKT = SKV // 128
    for b in range(B):
        idx = np.nonzero(keep[b])[0]
        n = len(idx)
        kTc = np.zeros((D, SKV), np.float32)
        kTc[:, :n] = k[b][idx].T
        vTc = np.zeros((D, SKV), np.float32)
        vTc[:, :n] = v[b][idx].T
        mvec = np.zeros(SKV, np.float32)
        mvec[:n] = 1.0
        qTb = np.ascontiguousarray(q[b].T).astype(mmd)
        kTc = kTc.astype(mmd)
        vTc = vTc.astype(mmd)
        for half in range(2):
            hs = slice(DH * half, DH * (half + 1))
            # smalls: col 0..2 bq dt-tiles, 3..5 bk, 6.. mv k-tiles
            sm = np.zeros((128, 6 + NKT), np.float32)
            sm[:, 0:3] = bq[hs].reshape(3, 128).T
            sm[:, 3:6] = bk[hs].reshape(3, 128).T
            sm[:, 6:] = mvec.reshape(NKT, 128).T
            in_maps.append(
                {
                    "qT": qTb,
                    "kT": kTc,
                    "vT": vTc,
                    "wq": np.ascontiguousarray(Wq[:, hs]).astype(mmd),
                    "wk": np.ascontiguousarray(Wk[:, hs]).astype(mmd),
                    "wv": np.ascontiguousarray(Wv[:, hs]).astype(mmd),
                    "wo": np.ascontiguousarray(Wo[hs, :]).astype(mmd),
                    "smalls": sm,
                }
            )
    return in_maps, SKV


def kernel(q, k, v, mask, Wq, bq, Wk, bk, Wv, bv, Wo, bo):
    q = np.asarray(q, np.float32)
    B, S, _ = q.shape
    bf16 = os.environ.get("BASS_PRECISE") != "1"
    in_maps, SKV = make_in_maps(q, k, v, mask, Wq, bq, Wk, bk, Wv, Wo, bf16=bf16)
    res = run_cores(
        in_maps, S=S, SKV=SKV, bf16=bf16,
        profile=os.environ.get("BASS_PROFILE") == "1",
    )
    if os.environ.get("BASS_PROFILE") == "1" and res.exec_time_ns is not None:
        print(f"HW exec time: {res.exec_time_ns} ns")
    cvec = (
        np.asarray(bv, np.float32) @ np.asarray(Wo, np.float32)
        + np.asarray(bo, np.float32)
    )
    out = np.empty((B, S, D), np.float32)
    for b in range(B):
        out[b] = (
            np.asarray(res.results[2 * b]["out"], np.float32)
            + np.asarray(res.results[2 * b + 1]["out"], np.float32)
            + cvec
        )
    return out


# revision 69
# speedup vs baseline: 1.0197x; 1.0197x over previous
"""MultiHeadAttention TRN2 Bass kernel.

Problem: B=4, S=2048, D=768, H=12 heads (DK=64).
Sharding: 8 cores = (batch b in 0..3) x (head-half in 0..1); each core
computes 6 heads of one batch element end-to-end (tensor-parallel over
heads within a batch). Host pre-transposes activations to [D, S] (and
casts to bf16 in the default fast path), slices projection weights per
head-half, and sums the two partial outputs per batch (+ bv@Wo + bo
correction, exact because softmax rows sum to 1).

Key optimization vs the dense formulation: the mask is per-key (same
for every query/head in a batch), so masked keys are removed ENTIRELY
on the host -- k/v are gathered down to the ~50% kept keys and padded
with zeros to SKV (multiple of 128, >= 1024). mv[s]=1 marks real keys,
0 marks padding; it is folded into vh_aug so padded keys contribute
exactly 0 to both the softmax numerator and denominator. This cuts
k/v-proj, scores, exp, and attn@V work by ~44% with bit-identical
semantics to the -inf mask.

On-core math (SKV = padded kept-key count, NKT = SKV/128):
  qh^T[384, S]: lhsT=Wq tile [Din,dout], rhs=q^T tile [Din,s] (+bq)
  kh^T[384, SKV] likewise; vh natural [SKV, 390] via lhsT=v^T, rhs=Wv:
    vh_aug[s, 65j..65j+64] = [mv(s)*vh_head_j(s, :), mv(s)]
  S^T[k, q] = kh_head^T.T @ qh_head^T  (contraction d=64; the two heads
    of a pair land on disjoint PE row quadrants -> they co-execute, and
    share one [128,1024] PSUM tile -> one exp per step)
  P^T = exp(S^T * 0.125)               (ACT, fused scale, no max-sub)
  ctx_aug^T[0:65, q] += vh_aug_j[kc].T @ P^T[kc]  over NKT k-chunks
    rows 0..63 = unnormalized ctx^T, row 64 = softmax denominator
  rs = reciprocal_approx_fast(denom copied to SBUF); bcast on gpsimd;
  cn = ctx^T * rs   (drains deferred so the PE pipeline never waits)
  out[q, 768] = sum_dt cn[dt].T @ Wo tiles  (per 128-q chunk), bf16 out

Scheduling notes (why it runs at ~194us instead of the 394us of the
dense ancestor):
- masked-key compaction cuts scores/exp/attn@V/kv-proj by ~44%
- phase 2 is co-bound: PE ~753ns/step + fillers vs ACT exp ~1.1us/step;
  deferred q-proj and O-proj run as fillers in fixed kc slots, placed
  >=4 steps after the drain that produces their input (in-order PE: a
  stalled filler blocks everything behind it)
- every dma_start rides ONE ~20GB/s hardware ring: big loads are split
  into ~100-150KB pieces issued round-robin over the sync/gpsimd/scalar
  queue sequencers (~600ns issue cost each), in consumption order
- the PE ramps 0.65->1.2->2.4GHz over ~3us of continuous work and any
  idle gap resets it: keep-warm dummy matmuls bridge the final drain
  wait so the tail O-projs run at full clock
- the first two scores+exp are hoisted before v-proj so the scalar
  engine's 108-exp stream starts as early as possible
"""

import os
import sys
import types
from contextlib import ExitStack

import ml_dtypes
import numpy as np

import concourse.bacc as bacc
import concourse.bass as bass
import concourse.mybir as mybir
import concourse.tile as tile
from concourse import bass_utils
from concourse.bass import ts, ds

F32 = mybir.dt.float32
F32R = mybir.dt.float32r
BF16 = mybir.dt.bfloat16

D = 768        # model dim
DH = 384       # per-core head dim (6 heads x 64)
HPC = 6        # heads per core
VW = HPC * 65  # vh_aug free width (390)


def build_nc(S=2048, SKV=1152, bf16=True):
    nc = bacc.Bacc("TRN2", target_bir_lowering=False, debug=False)

    MMD = BF16 if bf16 else F32R    # matmul operand dtype
    NKT = SKV // 128                # 128-wide k-tiles
    assert SKV % 128 == 0 and NKT >= 8
    QBW = min(512, S)               # attention q-block width
    NQB = S // QBW                  # q blocks
    CWQ = min(1024, S)              # q-proj s-chunk width
    # k-proj free-dim chunk: largest 128*d <= 512 with d | NKT
    CWK = next(128 * d for d in (4, 3, 2, 1) if NKT % d == 0)

    qT = nc.dram_tensor("qT", [D, S], MMD, kind="ExternalInput").ap()
    kT = nc.dram_tensor("kT", [D, SKV], MMD, kind="ExternalInput").ap()
    vT = nc.dram_tensor("vT", [D, SKV], MMD, kind="ExternalInput").ap()
    wq = nc.dram_tensor("wq", [D, DH], MMD, kind="ExternalInput").ap()
    wk = nc.dram_tensor("wk", [D, DH], MMD, kind="ExternalInput").ap()
    wv = nc.dram_tensor("wv", [D, DH], MMD, kind="ExternalInput").ap()
    wo = nc.dram_tensor("wo", [DH, D], MMD, kind="ExternalInput").ap()
    # col 0..2 = bq (3 dt-tiles), 3..5 = bk, 6..6+NKT = mv (padding flag)
    smalls = nc.dram_tensor("smalls", [128, 6 + NKT], F32, kind="ExternalInput").ap()
    out = nc.dram_tensor("out", [S, D], BF16, kind="ExternalOutput").ap()

    with tile.TileContext(nc) as tc, ExitStack() as ctx:
        P = 128
        wpool = ctx.enter_context(tc.tile_pool(name="w", bufs=1))
        xin = ctx.enter_context(tc.tile_pool(name="xin", bufs=12))
        persist = ctx.enter_context(tc.tile_pool(name="persist", bufs=1))
        ppool = ctx.enter_context(tc.tile_pool(name="p", bufs=3))
        small = ctx.enter_context(tc.tile_pool(name="small", bufs=2))
        outp = ctx.enter_context(tc.tile_pool(name="outp", bufs=2))
        psA = ctx.enter_context(tc.tile_pool(name="psA", bufs=2, space="PSUM"))
        psB = ctx.enter_context(tc.tile_pool(name="psB", bufs=4, space="PSUM"))

        # Round-robin DMA issue across 4 engine sequencers: each dma_start
        # costs ~600ns of issue time on its engine, so spreading the ~45
        # phase-1 loads over 4 queues (in dependency order: wk+kt first)
        # cuts the serial descriptor-issue head from ~20us to ~4us.
        # The scalar (ACT) queue must drain before exp(0) can dispatch --
        # every DMA issued on it costs ~600ns of sequencer time ahead of the
        # exp stream. So scalar only gets the earliest wave (wk/kt), and is
        # dropped from the rotation afterwards.
        dmaq = [nc.sync, nc.gpsimd, nc.scalar]
        dqi = [0]

        def dq_start(dst, src):
            dmaq[dqi[0] % len(dmaq)].dma_start(dst, src)
            dqi[0] += 1

        # ---- constants / small tensors ----
        wq_sb = [wpool.tile([P, DH], MMD, name=f"wq{c}", tag=f"wq{c}") for c in range(6)]
        wk_sb = [wpool.tile([P, DH], MMD, name=f"wk{c}", tag=f"wk{c}") for c in range(6)]
        wv_sb = [wpool.tile([P, DH], MMD, name=f"wv{c}", tag=f"wv{c}") for c in range(6)]
        wo_sb = [wpool.tile([P, D], MMD, name=f"wo{c}", tag=f"wo{c}") for c in range(3)]
        sm_sb = wpool.tile([128, 6 + NKT], F32, tag="smalls")
        # DMA priority order = compute order: q-proj runs first (so its data
        # loads first), k-proj next (kt fully landed by then -> no mid-kproj
        # DMA stalls that would reset the PE p-state), v/o/deferred-q last.
        # Each dma_start rides a single ~20GB/s hardware ring, so big loads
        # are split into ~128-150KB pieces to spread across the 16 rings,
        # issued in the order compute consumes them (kproj, vproj, qproj).
        # kt/vt pieces are split on the column boundaries the consuming
        # matmul chains use, and issued chunk-0-of-every-tile first, so the
        # first kproj/vproj chains start as early as possible.
        kt = [xin.tile([P, SKV], MMD, name="xin", tag="xin") for c in range(6)]
        for c in range(6):
            dq_start(wk_sb[c][:], wk[ts(c, P), :])
        for sc in range(SKV // CWK):
            for c in range(6):
                dq_start(kt[c][:, ts(sc, CWK)], kT[ts(c, P), ts(sc, CWK)])
        dmaq.pop()  # scalar queue must be free well before the first exp
        dq_start(sm_sb[:], smalls[:, :])
        bq_sb = [sm_sb[:, t : t + 1] for t in range(3)]
        bk_sb = [sm_sb[:, 3 + t : 4 + t] for t in range(3)]
        mv_sb = [sm_sb[:, 6 + st : 7 + st] for st in range(NKT)]
        ones6 = wpool.tile([P, HPC], F32, tag="ones6")
        nc.vector.memset(ones6[:], 1.0)
        qt0 = [xin.tile([P, CWQ], MMD, name="xin", tag="xin") for c in range(6)]
        for c in range(6):
            dq_start(wq_sb[c][:], wq[ts(c, P), :])
            dq_start(qt0[c][:, 0:512], qT[ts(c, P), 0:512])
        vt = [xin.tile([P, SKV], MMD, name="xin", tag="xin") for c in range(6)]
        for c in range(6):
            dq_start(wv_sb[c][:], wv[ts(c, P), :])
        for sc in range(SKV // CWK):
            for c in range(6):
                dq_start(vt[c][:, ts(sc, CWK)], vT[ts(c, P), ts(sc, CWK)])
        for c in range(6):
            dq_start(qt0[c][:, 512:CWQ], qT[ts(c, P), 512:CWQ])
        for c in range(3):
            dq_start(wo_sb[c][:], wo[ts(c, P), :])
        qproj_xt = {0: qt0}
        for sc in range(1, S // CWQ):
            qproj_xt[sc] = [
                xin.tile([P, CWQ], MMD, name="xin", tag="xin") for c in range(6)
            ]
            for c in range(6):
                dq_start(qproj_xt[sc][c][:, 0:512], qT[ts(c, P), ds(sc * CWQ, 512)])
                dq_start(
                    qproj_xt[sc][c][:, 512:CWQ],
                    qT[ts(c, P), ds(sc * CWQ + 512, 512)],
                )

        # ---- persistent activations ----
        khT = [persist.tile([P, SKV], MMD, name=f"khT{t}", tag=f"khT{t}") for t in range(3)]
        qhT = [persist.tile([P, S], MMD, name=f"qhT{t}", tag=f"qhT{t}") for t in range(3)]
        vh = [persist.tile([P, VW], MMD, name=f"vh{st}", tag=f"vh{st}") for st in range(NKT)]
        cn = [persist.tile([P, S], MMD, name=f"cn{t}", tag=f"cn{t}") for t in range(3)]

        # ---- phase 1: k-proj, v-proj, then q-proj of the first 512 cols
        # (matching DMA arrival order); the rest of q-proj is phase-2 filler.
        for sc in range(SKV // CWK):
            for dt in range(3):
                ps = psA.tile([P, CWK], F32, name="psA", tag="psA")
                for c in range(6):
                    nc.tensor.matmul(
                        ps[:],
                        lhsT=wk_sb[c][:, ts(dt, P)],
                        rhs=kt[c][:, ts(sc, CWK)],
                        start=(c == 0),
                        stop=(c == 5),
                    )
                nc.vector.tensor_scalar_add(
                    out=khT[dt][:, ts(sc, CWK)], in0=ps[:],
                    scalar1=bk_sb[dt],
                )

        def vproj_sub(st):
            ps = psB.tile([P, 512], F32, name="psB", tag="psB")
            for c in range(6):
                nc.tensor.matmul(
                    ps[:, :DH],
                    lhsT=vt[c][:, ts(st, P)],
                    rhs=wv_sb[c][:],
                    start=(c == 0),
                    stop=(c == 5),
                )
            vh3 = vh[st].rearrange("p (h c) -> p h c", c=65)
            nc.vector.tensor_scalar_mul(
                out=vh3[:, :, 0:64],
                in0=ps[:, :DH].rearrange("p (h c) -> p h c", c=64),
                scalar1=mv_sb[st],
            )
            nc.vector.tensor_scalar_mul(
                out=vh3[:, :, 64:65],
                in0=ones6[:].rearrange("p (h c) -> p h c", c=1),
                scalar1=mv_sb[st],
            )

        for dt in range(3):
            ps = psA.tile([P, 512], F32, name="psA", tag="psA")
            for c in range(6):
                nc.tensor.matmul(
                    ps[:],
                    lhsT=wq_sb[c][:, ts(dt, P)],
                    rhs=qt0[c][:, ts(0, 512)],
                    start=(c == 0),
                    stop=(c == 5),
                )
            nc.vector.tensor_scalar_add(
                out=qhT[dt][:, ts(0, 512)], in0=ps[:],
                scalar1=bq_sb[dt],
            )
        pend_qproj = [(0, dt, 1) for dt in range(3)] + [
            (sc, dt, u)
            for sc in range(1, S // CWQ)
            for dt in range(3)
            for u in range(CWQ // 512)
        ]

        # ---- phase 2: attention, head-pair steps ----
        # Each step handles BOTH heads of a pair for one k-chunk: the two
        # scores matmuls live in disjoint PE row groups (base partition 0
        # and 64) and share one [128,1024] PSUM tile (head A in cols 0:512,
        # head B in 512:1024) -> one exp per step. Scores run 2 steps ahead
        # of attn@V. Drains and filler work (deferred q-proj, O-proj) go to
        # psB spare slots, never stalling the scores pipeline on psA.
        hq = [(pr, qb) for qb in range(NQB) for pr in range(3)]
        steps = [(pr, qb, kc) for (pr, qb) in hq for kc in range(NKT)]

        ctx_ps = {}
        st_ps = {}

        def scores(pr, qb, kc):
            ps = psA.tile([P, 1024], F32, name="psA", tag="psA")
            for hh in range(2):
                nc.tensor.matmul(
                    ps[:, ts(hh, 512)],
                    lhsT=khT[pr][64 * hh : 64 * hh + 64, ts(kc, P)],
                    rhs=qhT[pr][64 * hh : 64 * hh + 64, ts(qb, QBW)],
                    start=True,
                    stop=True,
                )
            st_ps[(pr, qb, kc)] = ps

        # Hoist the first DEPTH scores ahead of v-proj: they only need
        # khT/qhT, so the scalar engine starts the exp stream while the PE
        # is still working through v-proj.
        scores(*steps[0])
        scores(*steps[1])
        for st in range(NKT):
            vproj_sub(st)

        def attnv(pr, qb, kc, pt):
            for hh in range(2):
                h = 2 * pr + hh
                nc.tensor.matmul(
                    ctx_ps[(h, qb)][0:65, :],
                    lhsT=vh[kc][:, ds(65 * h, 65)],
                    rhs=pt[:, ts(hh, 512)],
                    start=(kc == 0),
                    stop=(kc == NKT - 1),
                )

        def drain(h, qb):
            """Normalize + store ctx for a finished (h, qb)."""
            dt, pb = h // 2, 64 * (h % 2)
            cps = ctx_ps.pop((h, qb))
            den = small.tile([1, QBW], F32, name="den", tag="den")
            nc.vector.tensor_copy(den[:], cps[64:65, :])
            rs = small.tile([1, QBW], F32, name="rs", tag="rs")
            nc.vector.reciprocal_approx_fast(rs[:], den[:])
            bcs = small.tile([64, QBW], F32, name="bcs", tag="bcs")
            nc.gpsimd.partition_broadcast(bcs[:], rs[:])
            if pb == 0:
                nc.vector.tensor_tensor(
                    out=cn[dt][0:64, ts(qb, QBW)],
                    in0=cps[0:64, :],
                    in1=bcs[:],
                    op=mybir.AluOpType.mult,
                )
            else:
                tmp = small.tile([64, QBW], MMD, name="tmp", tag="tmp")
                nc.vector.tensor_tensor(
                    out=tmp[:], in0=cps[0:64, :], in1=bcs[:],
                    op=mybir.AluOpType.mult,
                )
                nc.gpsimd.dma_start(cn[dt][64:128, ts(qb, QBW)], tmp[:])

        oq = [nc.sync, nc.gpsimd]

        # Fillers (deferred q-proj, O-proj) are split into <=3-matmul micro
        # pieces chained via open PSUM accumulation, dispatched ONE per step:
        # a chunky filler between two scores delays the next exp (the phase-2
        # pacer) by its full length, costing ~2us per occurrence. Entries are
        # (allocates_psB, chain_len, fn); allocations only happen at kc >= 4
        # (after the previous group's ctx tiles are released) and early
        # enough that the chain closes within its group.
        pend_fill = []

        def queue_qproj(sc, dt, u):
            box = {}

            def p1():
                box["ps"] = psB.tile([P, 512], F32, name="psB", tag="psB")
                for c in range(3):
                    nc.tensor.matmul(
                        box["ps"][:],
                        lhsT=wq_sb[c][:, ts(dt, P)],
                        rhs=qproj_xt[sc][c][:, ts(u, 512)],
                        start=(c == 0),
                        stop=False,
                    )

            def p2():
                for c in range(3, 6):
                    nc.tensor.matmul(
                        box["ps"][:],
                        lhsT=wq_sb[c][:, ts(dt, P)],
                        rhs=qproj_xt[sc][c][:, ts(u, 512)],
                        start=False,
                        stop=(c == 5),
                    )
                nc.vector.tensor_scalar_add(
                    out=qhT[dt][:, ds(sc * CWQ + u * 512, 512)],
                    in0=box["ps"][:], scalar1=bq_sb[dt],
                )

            pend_fill.append((True, 2, p1))
            pend_fill.append((False, 0, p2))

        def queue_oproj(qc):
            box = {}

            def a1():
                box["ups"] = psB.tile([P, 512], F32, name="psB", tag="psB")
                for dt in range(2):
                    nc.tensor.matmul(
                        box["ups"][:],
                        lhsT=cn[dt][:, ts(qc, P)],
                        rhs=wo_sb[dt][:, ds(0, 512)],
                        start=(dt == 0),
                        stop=False,
                    )

            def a2():
                nc.tensor.matmul(
                    box["ups"][:],
                    lhsT=cn[2][:, ts(qc, P)],
                    rhs=wo_sb[2][:, ds(0, 512)],
                    start=False,
                    stop=True,
                )
                box["ot"] = outp.tile([P, D], MMD, name="ot", tag="ot")
                nc.vector.tensor_copy(box["ot"][:, 0:512], box["ups"][:, 0:512])
                oq[qc % len(oq)].dma_start(
                    out[ts(qc, P), 0:512], box["ot"][:, 0:512]
                )

            def b1():
                box["ups2"] = psB.tile([P, 256], F32, name="psB2", tag="psB")
                for dt in range(2):
                    nc.tensor.matmul(
                        box["ups2"][:, 0:256],
                        lhsT=cn[dt][:, ts(qc, P)],
                        rhs=wo_sb[dt][:, ds(512, 256)],
                        start=(dt == 0),
                        stop=False,
                    )

            def b2():
                nc.tensor.matmul(
                    box["ups2"][:, 0:256],
                    lhsT=cn[2][:, ts(qc, P)],
                    rhs=wo_sb[2][:, ds(512, 256)],
                    start=False,
                    stop=True,
                )
                nc.vector.tensor_copy(box["ot"][:, 512:768], box["ups2"][:, 0:256])
                oq[(qc + 1) % len(oq)].dma_start(
                    out[ts(qc, P), 512:768], box["ot"][:, 512:768]
                )

            pend_fill.append((True, 2, a1))
            pend_fill.append((False, 0, a2))
            pend_fill.append((True, 2, b1))
            pend_fill.append((False, 0, b2))

        def queue_vproj(st):
            box = {}

            def v1():
                box["ps"] = psB.tile([P, 512], F32, name="psB", tag="psB")
                for c in range(3):
                    nc.tensor.matmul(
                        box["ps"][:, :DH],
                        lhsT=vt[c][:, ts(st, P)],
                        rhs=wv_sb[c][:],
                        start=(c == 0),
                        stop=False,
                    )

            def v2():
                for c in range(3, 6):
                    nc.tensor.matmul(
                        box["ps"][:, :DH],
                        lhsT=vt[c][:, ts(st, P)],
                        rhs=wv_sb[c][:],
                        start=False,
                        stop=(c == 5),
                    )
                vh3 = vh[st].rearrange("p (h c) -> p h c", c=65)
                nc.vector.tensor_scalar_mul(
                    out=vh3[:, :, 0:64],
                    in0=box["ps"][:, :DH].rearrange("p (h c) -> p h c", c=64),
                    scalar1=mv_sb[st],
                )
                nc.vector.tensor_scalar_mul(
                    out=vh3[:, :, 64:65],
                    in0=ones6[:].rearrange("p (h c) -> p h c", c=1),
                    scalar1=mv_sb[st],
                )

            pend_fill.append((True, 2, v1))
            pend_fill.append((False, 0, v2))

        for e in pend_qproj:
            queue_qproj(*e)

        DEPTH = 2
        pend_drain = []
        for n, (pr, qb, kc) in enumerate(steps):
            if kc == 0:
                for hh in range(2):
                    ctx_ps[(2 * pr + hh, qb)] = psB.tile(
                        [P, QBW], F32, name="psB", tag="psB"
                    )[0:65, :]
            pt = ppool.tile([P, 1024], MMD, name="pt", tag="pt")
            nc.scalar.activation(
                pt[:], st_ps.pop((pr, qb, kc))[:],
                mybir.ActivationFunctionType.Exp, scale=0.125,
            )
            if n + DEPTH < len(steps):
                scores(*steps[n + DEPTH])
            attnv(pr, qb, kc, pt)
            if kc in (0, 1) and pend_drain:
                hd, qd = pend_drain.pop(0)
                drain(hd, qd)
                if hd == HPC - 1:
                    for qcx in range(qd * (QBW // P), (qd + 1) * (QBW // P)):
                        queue_oproj(qcx)
            elif pend_fill:
                na, cl, fn = pend_fill[0]
                if (not na) or (4 <= kc <= NKT - cl):
                    pend_fill.pop(0)
                    fn()
            if kc == NKT - 1:
                pend_drain.extend([(2 * pr, qb), (2 * pr + 1, qb)])
        while pend_fill:
            pend_fill.pop(0)[2]()
        # tail: keep-warm matmuls bridge the drain-wait gap so the final
        # o-proj matmuls run at full PE clock instead of mid p-state
        wps = psA.tile([P, 512], F32, name="psA", tag="psA")
        for _ in range(26):
            nc.tensor.matmul(
                wps[:], lhsT=khT[0][:, 0:128], rhs=khT[0][:, 0:512],
                start=True, stop=True,
            )
        # batch the final drains phase-by-phase so the two DVE chains
        # and the two gpsimd broadcasts interleave instead of serializing
        infos = []
        for hd, qd in pend_drain:
            dt, pb = hd // 2, 64 * (hd % 2)
            cps = ctx_ps.pop((hd, qd))
            den = small.tile([1, QBW], F32, name="den", tag="den")
            nc.vector.tensor_copy(den[:], cps[64:65, :])
            rs = small.tile([1, QBW], F32, name="rs", tag="rs")
            nc.vector.reciprocal_approx_fast(rs[:], den[:])
            bcs = small.tile([64, QBW], F32, name="bcs", tag="bcs")
            nc.gpsimd.partition_broadcast(bcs[:], rs[:])
            infos.append((hd, qd, dt, pb, cps, bcs))
        for hd, qd, dt, pb, cps, bcs in infos:
            if pb == 0:
                nc.vector.tensor_tensor(
                    out=cn[dt][0:64, ts(qd, QBW)],
                    in0=cps[0:64, :], in1=bcs[:],
                    op=mybir.AluOpType.mult,
                )
            else:
                tmp = small.tile([64, QBW], MMD, name="tmp", tag="tmp")
                nc.vector.tensor_tensor(
                    out=tmp[:], in0=cps[0:64, :], in1=bcs[:],
                    op=mybir.AluOpType.mult,
                )
                nc.gpsimd.dma_start(cn[dt][64:128, ts(qd, QBW)], tmp[:])
            if hd == HPC - 1:
                for qcx in range(qd * (QBW // P), (qd + 1) * (QBW // P)):
                    queue_oproj(qcx)
        oq.append(nc.scalar)  # scalar queue is free once the exps are done
        while pend_fill:
            pend_fill.pop(0)[2]()

    nc.compile()
    return nc


_NC_CACHE = {}


def _get_nc(S, SKV, bf16=True):
    key = (S, SKV, bf16)
    if key not in _NC_CACHE:
        _NC_CACHE[key] = build_nc(S, SKV, bf16)
    return _NC_CACHE[key]


def _install_ntff_hook():
    try:
        mod = types.ModuleType("antenv.axon_hooks")
        state = {"hook": None}
        mod.set_axon_ntff_profile_hook = lambda h: state.__setitem__("hook", h)
        mod.get_axon_ntff_profile_hook = lambda: state["hook"]
        sys.modules["antenv.axon_hooks"] = mod
        from trn_agent_boot.trn_boot import _ntff_profile_via_ctypes

        mod.set_axon_ntff_profile_hook(
            _ntff_profile_via_ctypes("/opt/axon/libaxon_pjrt.so")
        )
        bass_utils.upload_artifacts = lambda tmpdir: "local://" + tmpdir
        return state["hook"] is not None
    except Exception:
        return False


def run_cores(in_maps, S=2048, SKV=1152, bf16=True, profile=False):
    nc = _get_nc(S, SKV, bf16)
    trace = bool(profile) and _install_ntff_hook()
    res = bass_utils.run_bass_kernel_spmd(
        nc, in_maps, core_ids=list(range(len(in_maps))), trace=trace
    )
    return res


def make_in_maps(q, k, v, mask, Wq, bq, Wk, bk, Wv, Wo, bf16=True):
    B, S, _ = q.shape
    mmd = ml_dtypes.bfloat16 if bf16 else np.float32
    q = np.asarray(q, np.float32)
    k = np.asarray(k, np.float32)
    v = np.asarray(v, np.float32)
    keep = ~np.asarray(mask).reshape(B, S)
    counts = keep.sum(axis=1)
    SKV = max(1024, int(-(-int(counts.max()) // 128)) * 128)
    Wq, Wk, Wv, Wo = (np.asarray(a, np.float32) for a in (Wq, Wk, Wv, Wo))
    bq, bk = np.asarray(bq, np.float32), np.asarray(bk, np.float32)
    in_maps = []
    NKT = SKV // 128
    for b in range(B):
        idx = np.nonzero(keep[b])[0]
        n = len(idx)
        kTc = np.zeros((D, SKV), np.float32)
        kTc[:, :n] = k[b][idx].T
        vTc = np.zeros((D, SKV), np.float32)
        vTc[:, :n] = v[b][idx].T
        mvec = np.zeros(SKV, np.float32)
        mvec[:n] = 1.0
        qTb = np.ascontiguousarray(q[b].T).astype(mmd)
        kTc = kTc.astype(mmd)
        vTc = vTc.astype(mmd)
        for half in range(2):
            hs = slice(DH * half, DH * (half + 1))
            # smalls: col 0..2 bq dt-tiles, 3..5 bk, 6.. mv k-tiles
            sm = np.zeros((128, 6 + NKT), np.float32)
            sm[:, 0:3] = bq[hs].reshape(3, 128).T
            sm[:, 3:6] = bk[hs].reshape(3, 128).T
            sm[:, 6:] = mvec.reshape(NKT, 128).T
            in_maps.append(
                {
                    "qT": qTb,
                    "kT": kTc,
                    "vT": vTc,
                    "wq": np.ascontiguousarray(Wq[:, hs]).astype(mmd),
                    "wk": np.ascontiguousarray(Wk[:, hs]).astype(mmd),
                    "wv": np.ascontiguousarray(Wv[:, hs]).astype(mmd),
                    "wo": np.ascontiguousarray(Wo[hs, :]).astype(mmd),
                    "smalls": sm,
                }
            )
    return in_maps, SKV


def kernel(q, k, v, mask, Wq, bq, Wk, bk, Wv, bv, Wo, bo):
    q = np.asarray(q, np.float32)
    B, S, _ = q.shape
    bf16 = os.environ.get("BASS_PRECISE") != "1"
    in_maps, SKV = make_in_maps(q, k, v, mask, Wq, bq, Wk, bk, Wv, Wo, bf16=bf16)
    res = run_cores(
        in_maps, S=S, SKV=SKV, bf16=bf16,
        profile=os.environ.get("BASS_PROFILE") == "1",
    )
    if os.environ.get("BASS_PROFILE") == "1" and res.exec_time_ns is not None:
        print(f"HW exec time: {res.exec_time_ns} ns")
    cvec = (
        np.asarray(bv, np.float32) @ np.asarray(Wo, np.float32)
        + np.asarray(bo, np.float32)
    )
    out = np.empty((B, S, D), np.float32)
    for b in range(B):
        out[b] = (
            np.asarray(res.results[2 * b]["out"], np.float32)
            + np.asarray(res.results[2 * b + 1]["out"], np.float32)
            + cvec
        )
    return out


# revision 70
# speedup vs baseline: 1.0207x; 1.0010x over previous
"""MultiHeadAttention TRN2 Bass kernel.

Problem: B=4, S=2048, D=768, H=12 heads (DK=64).
Sharding: 8 cores = (batch b in 0..3) x (head-half in 0..1); each core
computes 6 heads of one batch element end-to-end (tensor-parallel over
heads within a batch). Host pre-transposes activations to [D, S] (and
casts to bf16 in the default fast path), slices projection weights per
head-half, and sums the two partial outputs per batch (+ bv@Wo + bo
correction, exact because softmax rows sum to 1).

Key optimization vs the dense formulation: the mask is per-key (same
for every query/head in a batch), so masked keys are removed ENTIRELY
on the host -- k/v are gathered down to the ~50% kept keys and padded
with zeros to SKV (multiple of 128, >= 1024). mv[s]=1 marks real keys,
0 marks padding; it is folded into vh_aug so padded keys contribute
exactly 0 to both the softmax numerator and denominator. This cuts
k/v-proj, scores, exp, and attn@V work by ~44% with bit-identical
semantics to the -inf mask.

On-core math (SKV = padded kept-key count, NKT = SKV/128):
  qh^T[384, S]: lhsT=Wq tile [Din,dout], rhs=q^T tile [Din,s] (+bq)
  kh^T[384, SKV] likewise; vh natural [SKV, 390] via lhsT=v^T, rhs=Wv:
    vh_aug[s, 65j..65j+64] = [mv(s)*vh_head_j(s, :), mv(s)]
  S^T[k, q] = kh_head^T.T @ qh_head^T  (contraction d=64; the two heads
    of a pair land on disjoint PE row quadrants -> they co-execute, and
    share one [128,1024] PSUM tile -> one exp per step)
  P^T = exp(S^T * 0.125)               (ACT, fused scale, no max-sub)
  ctx_aug^T[0:65, q] += vh_aug_j[kc].T @ P^T[kc]  over NKT k-chunks
    rows 0..63 = unnormalized ctx^T, row 64 = softmax denominator
  rs = reciprocal_approx_fast(denom copied to SBUF); bcast on gpsimd;
  cn = ctx^T * rs   (drains deferred so the PE pipeline never waits)
  out[q, 768] = sum_dt cn[dt].T @ Wo tiles  (per 128-q chunk), bf16 out

Scheduling notes (why it runs at ~194us instead of the 394us of the
dense ancestor):
- masked-key compaction cuts scores/exp/attn@V/kv-proj by ~44%
- phase 2 is co-bound: PE ~753ns/step + fillers vs ACT exp ~1.1us/step;
  deferred q-proj and O-proj run as fillers in fixed kc slots, placed
  >=4 steps after the drain that produces their input (in-order PE: a
  stalled filler blocks everything behind it)
- every dma_start rides ONE ~20GB/s hardware ring: big loads are split
  into ~100-150KB pieces issued round-robin over the sync/gpsimd/scalar
  queue sequencers (~600ns issue cost each), in consumption order
- the PE ramps 0.65->1.2->2.4GHz over ~3us of continuous work and any
  idle gap resets it: keep-warm dummy matmuls bridge the final drain
  wait so the tail O-projs run at full clock
- the first two scores+exp are hoisted before v-proj so the scalar
  engine's 108-exp stream starts as early as possible
"""

import os
import sys
import types
from contextlib import ExitStack

import ml_dtypes
import numpy as np

import concourse.bacc as bacc
import concourse.bass as bass
import concourse.mybir as mybir
import concourse.tile as tile
from concourse import bass_utils
from concourse.bass import ts, ds

F32 = mybir.dt.float32
F32R = mybir.dt.float32r
BF16 = mybir.dt.bfloat16

D = 768        # model dim
DH = 384       # per-core head dim (6 heads x 64)
HPC = 6        # heads per core
VW = HPC * 65  # vh_aug free width (390)


def build_nc(S=2048, SKV=1152, bf16=True):
    nc = bacc.Bacc("TRN2", target_bir_lowering=False, debug=False)

    MMD = BF16 if bf16 else F32R    # matmul operand dtype
    NKT = SKV // 128                # 128-wide k-tiles
    assert SKV % 128 == 0 and NKT >= 8
    QBW = min(512, S)               # attention q-block width
    NQB = S // QBW                  # q blocks
    CWQ = min(1024, S)              # q-proj s-chunk width
    # k-proj free-dim chunk: largest 128*d <= 512 with d | NKT
    CWK = next(128 * d for d in (4, 3, 2, 1) if NKT % d == 0)

    qT = nc.dram_tensor("qT", [D, S], MMD, kind="ExternalInput").ap()
    kT = nc.dram_tensor("kT", [D, SKV], MMD, kind="ExternalInput").ap()
    vT = nc.dram_tensor("vT", [D, SKV], MMD, kind="ExternalInput").ap()
    wq = nc.dram_tensor("wq", [D, DH], MMD, kind="ExternalInput").ap()
    wk = nc.dram_tensor("wk", [D, DH], MMD, kind="ExternalInput").ap()
    wv = nc.dram_tensor("wv", [D, DH], MMD, kind="ExternalInput").ap()
    wo = nc.dram_tensor("wo", [DH, D], MMD, kind="ExternalInput").ap()
    # col 0..2 = bq (3 dt-tiles), 3..5 = bk, 6..6+NKT = mv (padding flag)
    smalls = nc.dram_tensor("smalls", [128, 6 + NKT], F32, kind="ExternalInput").ap()
    out = nc.dram_tensor("out", [S, D], BF16, kind="ExternalOutput").ap()

    with tile.TileContext(nc) as tc, ExitStack() as ctx:
        P = 128
        wpool = ctx.enter_context(tc.tile_pool(name="w", bufs=1))
        xin = ctx.enter_context(tc.tile_pool(name="xin", bufs=12))
        persist = ctx.enter_context(tc.tile_pool(name="persist", bufs=1))
        ppool = ctx.enter_context(tc.tile_pool(name="p", bufs=3))
        small = ctx.enter_context(tc.tile_pool(name="small", bufs=2))
        outp = ctx.enter_context(tc.tile_pool(name="outp", bufs=2))
        psA = ctx.enter_context(tc.tile_pool(name="psA", bufs=2, space="PSUM"))
        psB = ctx.enter_context(tc.tile_pool(name="psB", bufs=4, space="PSUM"))

        # Round-robin DMA issue across 4 engine sequencers: each dma_start
        # costs ~600ns of issue time on its engine, so spreading the ~45
        # phase-1 loads over 4 queues (in dependency order: wk+kt first)
        # cuts the serial descriptor-issue head from ~20us to ~4us.
        # The scalar (ACT) queue must drain before exp(0) can dispatch --
        # every DMA issued on it costs ~600ns of sequencer time ahead of the
        # exp stream. So scalar only gets the earliest wave (wk/kt), and is
        # dropped from the rotation afterwards.
        dmaq = [nc.sync, nc.gpsimd, nc.scalar]
        dqi = [0]

        def dq_start(dst, src):
            dmaq[dqi[0] % len(dmaq)].dma_start(dst, src)
            dqi[0] += 1

        # ---- constants / small tensors ----
        wq_sb = [wpool.tile([P, DH], MMD, name=f"wq{c}", tag=f"wq{c}") for c in range(6)]
        wk_sb = [wpool.tile([P, DH], MMD, name=f"wk{c}", tag=f"wk{c}") for c in range(6)]
        wv_sb = [wpool.tile([P, DH], MMD, name=f"wv{c}", tag=f"wv{c}") for c in range(6)]
        wo_sb = [wpool.tile([P, D], MMD, name=f"wo{c}", tag=f"wo{c}") for c in range(3)]
        sm_sb = wpool.tile([128, 6 + NKT], F32, tag="smalls")
        # DMA priority order = compute order: q-proj runs first (so its data
        # loads first), k-proj next (kt fully landed by then -> no mid-kproj
        # DMA stalls that would reset the PE p-state), v/o/deferred-q last.
        # Each dma_start rides a single ~20GB/s hardware ring, so big loads
        # are split into ~128-150KB pieces to spread across the 16 rings,
        # issued in the order compute consumes them (kproj, vproj, qproj).
        # kt/vt pieces are split on the column boundaries the consuming
        # matmul chains use, and issued chunk-0-of-every-tile first, so the
        # first kproj/vproj chains start as early as possible.
        kt = [xin.tile([P, SKV], MMD, name="xin", tag="xin") for c in range(6)]
        for c in range(6):
            dq_start(wk_sb[c][:], wk[ts(c, P), :])
        for sc in range(SKV // CWK):
            for c in range(6):
                dq_start(kt[c][:, ts(sc, CWK)], kT[ts(c, P), ts(sc, CWK)])
        dmaq.pop()  # scalar queue must be free well before the first exp
        dq_start(sm_sb[:], smalls[:, :])
        bq_sb = [sm_sb[:, t : t + 1] for t in range(3)]
        bk_sb = [sm_sb[:, 3 + t : 4 + t] for t in range(3)]
        mv_sb = [sm_sb[:, 6 + st : 7 + st] for st in range(NKT)]
        ones6 = wpool.tile([P, HPC], F32, tag="ones6")
        nc.vector.memset(ones6[:], 1.0)
        qt0 = [xin.tile([P, CWQ], MMD, name="xin", tag="xin") for c in range(6)]
        for c in range(6):
            dq_start(wq_sb[c][:], wq[ts(c, P), :])
            dq_start(qt0[c][:, 0:512], qT[ts(c, P), 0:512])
        vt = [xin.tile([P, SKV], MMD, name="xin", tag="xin") for c in range(6)]
        for c in range(6):
            dq_start(wv_sb[c][:], wv[ts(c, P), :])
        for sc in range(SKV // CWK):
            for c in range(6):
                dq_start(vt[c][:, ts(sc, CWK)], vT[ts(c, P), ts(sc, CWK)])
        for c in range(6):
            dq_start(qt0[c][:, 512:CWQ], qT[ts(c, P), 512:CWQ])
        for c in range(3):
            dq_start(wo_sb[c][:], wo[ts(c, P), :])
        qproj_xt = {0: qt0}
        for sc in range(1, S // CWQ):
            qproj_xt[sc] = [
                xin.tile([P, CWQ], MMD, name="xin", tag="xin") for c in range(6)
            ]
            for c in range(6):
                dq_start(qproj_xt[sc][c][:, 0:512], qT[ts(c, P), ds(sc * CWQ, 512)])
                dq_start(
                    qproj_xt[sc][c][:, 512:CWQ],
                    qT[ts(c, P), ds(sc * CWQ + 512, 512)],
                )

        # ---- persistent activations ----
        khT = [persist.tile([P, SKV], MMD, name=f"khT{t}", tag=f"khT{t}") for t in range(3)]
        qhT = [persist.tile([P, S], MMD, name=f"qhT{t}", tag=f"qhT{t}") for t in range(3)]
        vh = [persist.tile([P, VW], MMD, name=f"vh{st}", tag=f"vh{st}") for st in range(NKT)]
        cn = [persist.tile([P, S], MMD, name=f"cn{t}", tag=f"cn{t}") for t in range(3)]

        # ---- phase 1: k-proj, v-proj, then q-proj of the first 512 cols
        # (matching DMA arrival order); the rest of q-proj is phase-2 filler.
        for sc in range(SKV // CWK):
            for dt in range(3):
                ps = psA.tile([P, CWK], F32, name="psA", tag="psA")
                for c in range(6):
                    nc.tensor.matmul(
                        ps[:],
                        lhsT=wk_sb[c][:, ts(dt, P)],
                        rhs=kt[c][:, ts(sc, CWK)],
                        start=(c == 0),
                        stop=(c == 5),
                    )
                nc.vector.tensor_scalar_add(
                    out=khT[dt][:, ts(sc, CWK)], in0=ps[:],
                    scalar1=bk_sb[dt],
                )

        def vproj_sub(st):
            ps = psB.tile([P, 512], F32, name="psB", tag="psB")
            for c in range(6):
                nc.tensor.matmul(
                    ps[:, :DH],
                    lhsT=vt[c][:, ts(st, P)],
                    rhs=wv_sb[c][:],
                    start=(c == 0),
                    stop=(c == 5),
                )
            vh3 = vh[st].rearrange("p (h c) -> p h c", c=65)
            nc.vector.tensor_scalar_mul(
                out=vh3[:, :, 0:64],
                in0=ps[:, :DH].rearrange("p (h c) -> p h c", c=64),
                scalar1=mv_sb[st],
            )
            nc.vector.tensor_scalar_mul(
                out=vh3[:, :, 64:65],
                in0=ones6[:].rearrange("p (h c) -> p h c", c=1),
                scalar1=mv_sb[st],
            )

        for dt in range(3):
            ps = psA.tile([P, 512], F32, name="psA", tag="psA")
            for c in range(6):
                nc.tensor.matmul(
                    ps[:],
                    lhsT=wq_sb[c][:, ts(dt, P)],
                    rhs=qt0[c][:, ts(0, 512)],
                    start=(c == 0),
                    stop=(c == 5),
                )
            nc.vector.tensor_scalar_add(
                out=qhT[dt][:, ts(0, 512)], in0=ps[:],
                scalar1=bq_sb[dt],
            )
        pend_qproj = [(0, dt, 1) for dt in range(3)] + [
            (sc, dt, u)
            for sc in range(1, S // CWQ)
            for dt in range(3)
            for u in range(CWQ // 512)
        ]

        # ---- phase 2: attention, head-pair steps ----
        # Each step handles BOTH heads of a pair for one k-chunk: the two
        # scores matmuls live in disjoint PE row groups (base partition 0
        # and 64) and share one [128,1024] PSUM tile (head A in cols 0:512,
        # head B in 512:1024) -> one exp per step. Scores run 2 steps ahead
        # of attn@V. Drains and filler work (deferred q-proj, O-proj) go to
        # psB spare slots, never stalling the scores pipeline on psA.
        hq = [(pr, qb) for qb in range(NQB) for pr in range(3)]
        steps = [(pr, qb, kc) for (pr, qb) in hq for kc in range(NKT)]

        ctx_ps = {}
        st_ps = {}

        def scores(pr, qb, kc):
            ps = psA.tile([P, 1024], F32, name="psA", tag="psA")
            for hh in range(2):
                nc.tensor.matmul(
                    ps[:, ts(hh, 512)],
                    lhsT=khT[pr][64 * hh : 64 * hh + 64, ts(kc, P)],
                    rhs=qhT[pr][64 * hh : 64 * hh + 64, ts(qb, QBW)],
                    start=True,
                    stop=True,
                )
            st_ps[(pr, qb, kc)] = ps

        # Hoist the first DEPTH scores ahead of v-proj: they only need
        # khT/qhT, so the scalar engine starts the exp stream while the PE
        # is still working through v-proj.
        scores(*steps[0])
        scores(*steps[1])
        for st in range(NKT):
            vproj_sub(st)

        def attnv(pr, qb, kc, pt):
            for hh in range(2):
                h = 2 * pr + hh
                nc.tensor.matmul(
                    ctx_ps[(h, qb)][0:65, :],
                    lhsT=vh[kc][:, ds(65 * h, 65)],
                    rhs=pt[:, ts(hh, 512)],
                    start=(kc == 0),
                    stop=(kc == NKT - 1),
                )

        def drain(h, qb):
            """Normalize + store ctx for a finished (h, qb)."""
            dt, pb = h // 2, 64 * (h % 2)
            cps = ctx_ps.pop((h, qb))
            den = small.tile([1, QBW], F32, name="den", tag="den")
            nc.vector.tensor_copy(den[:], cps[64:65, :])
            rs = small.tile([1, QBW], F32, name="rs", tag="rs")
            nc.vector.reciprocal_approx_fast(rs[:], den[:])
            bcs = small.tile([64, QBW], F32, name="bcs", tag="bcs")
            nc.gpsimd.partition_broadcast(bcs[:], rs[:])
            if pb == 0:
                nc.vector.tensor_tensor(
                    out=cn[dt][0:64, ts(qb, QBW)],
                    in0=cps[0:64, :],
                    in1=bcs[:],
                    op=mybir.AluOpType.mult,
                )
            else:
                tmp = small.tile([64, QBW], MMD, name="tmp", tag="tmp")
                nc.vector.tensor_tensor(
                    out=tmp[:], in0=cps[0:64, :], in1=bcs[:],
                    op=mybir.AluOpType.mult,
                )
                nc.gpsimd.dma_start(cn[dt][64:128, ts(qb, QBW)], tmp[:])

        oq = [nc.sync, nc.gpsimd]

        # Fillers (deferred q-proj, O-proj) are split into <=3-matmul micro
        # pieces chained via open PSUM accumulation, dispatched ONE per step:
        # a chunky filler between two scores delays the next exp (the phase-2
        # pacer) by its full length, costing ~2us per occurrence. Entries are
        # (allocates_psB, chain_len, fn); allocations only happen at kc >= 4
        # (after the previous group's ctx tiles are released) and early
        # enough that the chain closes within its group.
        pend_fill = []

        def queue_qproj(sc, dt, u):
            box = {}

            def p1():
                box["ps"] = psB.tile([P, 512], F32, name="psB", tag="psB")
                for c in range(3):
                    nc.tensor.matmul(
                        box["ps"][:],
                        lhsT=wq_sb[c][:, ts(dt, P)],
                        rhs=qproj_xt[sc][c][:, ts(u, 512)],
                        start=(c == 0),
                        stop=False,
                    )

            def p2():
                for c in range(3, 6):
                    nc.tensor.matmul(
                        box["ps"][:],
                        lhsT=wq_sb[c][:, ts(dt, P)],
                        rhs=qproj_xt[sc][c][:, ts(u, 512)],
                        start=False,
                        stop=(c == 5),
                    )
                nc.vector.tensor_scalar_add(
                    out=qhT[dt][:, ds(sc * CWQ + u * 512, 512)],
                    in0=box["ps"][:], scalar1=bq_sb[dt],
                )

            pend_fill.append((True, 2, p1))
            pend_fill.append((False, 0, p2))

        def queue_oproj(qc):
            box = {}

            def a1():
                box["ups"] = psB.tile([P, 512], F32, name="psB", tag="psB")
                for dt in range(2):
                    nc.tensor.matmul(
                        box["ups"][:],
                        lhsT=cn[dt][:, ts(qc, P)],
                        rhs=wo_sb[dt][:, ds(0, 512)],
                        start=(dt == 0),
                        stop=False,
                    )

            def a2():
                nc.tensor.matmul(
                    box["ups"][:],
                    lhsT=cn[2][:, ts(qc, P)],
                    rhs=wo_sb[2][:, ds(0, 512)],
                    start=False,
                    stop=True,
                )
                box["ot"] = outp.tile([P, D], MMD, name="ot", tag="ot")
                nc.vector.tensor_copy(box["ot"][:, 0:512], box["ups"][:, 0:512])
                oq[qc % len(oq)].dma_start(
                    out[ts(qc, P), 0:512], box["ot"][:, 0:512]
                )

            def b1():
                box["ups2"] = psB.tile([P, 256], F32, name="psB2", tag="psB")
                for dt in range(2):
                    nc.tensor.matmul(
                        box["ups2"][:, 0:256],
                        lhsT=cn[dt][:, ts(qc, P)],
                        rhs=wo_sb[dt][:, ds(512, 256)],
                        start=(dt == 0),
                        stop=False,
                    )

            def b2():
                nc.tensor.matmul(
                    box["ups2"][:, 0:256],
                    lhsT=cn[2][:, ts(qc, P)],
                    rhs=wo_sb[2][:, ds(512, 256)],
                    start=False,
                    stop=True,
                )
                nc.vector.tensor_copy(box["ot"][:, 512:768], box["ups2"][:, 0:256])
                oq[(qc + 1) % len(oq)].dma_start(
                    out[ts(qc, P), 512:768], box["ot"][:, 512:768]
                )

            pend_fill.append((True, 2, a1))
            pend_fill.append((False, 0, a2))
            pend_fill.append((True, 2, b1))
            pend_fill.append((False, 0, b2))

        def queue_vproj(st):
            box = {}

            def v1():
                box["ps"] = psB.tile([P, 512], F32, name="psB", tag="psB")
                for c in range(3):
                    nc.tensor.matmul(
                        box["ps"][:, :DH],
                        lhsT=vt[c][:, ts(st, P)],
                        rhs=wv_sb[c][:],
                        start=(c == 0),
                        stop=False,
                    )

            def v2():
                for c in range(3, 6):
                    nc.tensor.matmul(
                        box["ps"][:, :DH],
                        lhsT=vt[c][:, ts(st, P)],
                        rhs=wv_sb[c][:],
                        start=False,
                        stop=(c == 5),
                    )
                vh3 = vh[st].rearrange("p (h c) -> p h c", c=65)
                nc.vector.tensor_scalar_mul(
                    out=vh3[:, :, 0:64],
                    in0=box["ps"][:, :DH].rearrange("p (h c) -> p h c", c=64),
                    scalar1=mv_sb[st],
                )
                nc.vector.tensor_scalar_mul(
                    out=vh3[:, :, 64:65],
                    in0=ones6[:].rearrange("p (h c) -> p h c", c=1),
                    scalar1=mv_sb[st],
                )

            pend_fill.append((True, 2, v1))
            pend_fill.append((False, 0, v2))

        for e in pend_qproj:
            queue_qproj(*e)

        DEPTH = 2
        pend_drain = []
        for n, (pr, qb, kc) in enumerate(steps):
            if kc == 0:
                for hh in range(2):
                    ctx_ps[(2 * pr + hh, qb)] = psB.tile(
                        [P, QBW], F32, name="psB", tag="psB"
                    )[0:65, :]
            pt = ppool.tile([P, 1024], MMD, name="pt", tag="pt")
            nc.scalar.activation(
                pt[:], st_ps.pop((pr, qb, kc))[:],
                mybir.ActivationFunctionType.Exp, scale=0.125,
            )
            if n + DEPTH < len(steps):
                scores(*steps[n + DEPTH])
            attnv(pr, qb, kc, pt)
            if kc in (0, 1) and pend_drain:
                hd, qd = pend_drain.pop(0)
                drain(hd, qd)
                if hd == HPC - 1:
                    for qcx in range(qd * (QBW // P), (qd + 1) * (QBW // P)):
                        queue_oproj(qcx)
            elif pend_fill:
                na, cl, fn = pend_fill[0]
                if (not na) or (4 <= kc <= NKT - cl):
                    pend_fill.pop(0)
                    fn()
            if kc == NKT - 1:
                pend_drain.extend([(2 * pr, qb), (2 * pr + 1, qb)])
        while pend_fill:
            pend_fill.pop(0)[2]()
        # tail: keep-warm matmuls bridge the drain-wait gap so the final
        # o-proj matmuls run at full PE clock instead of mid p-state
        wps = psA.tile([P, 512], F32, name="psA", tag="psA")
        for _ in range(16):
            nc.tensor.matmul(
                wps[:], lhsT=khT[0][:, 0:128], rhs=khT[0][:, 0:512],
                start=True, stop=True,
            )
        # batch the final drains phase-by-phase so the two DVE chains
        # and the two gpsimd broadcasts interleave instead of serializing
        infos = []
        for hd, qd in pend_drain:
            dt, pb = hd // 2, 64 * (hd % 2)
            cps = ctx_ps.pop((hd, qd))
            den = small.tile([1, QBW], F32, name="den", tag="den")
            nc.vector.tensor_copy(den[:], cps[64:65, :])
            rs = small.tile([1, QBW], F32, name="rs", tag="rs")
            nc.vector.reciprocal_approx_fast(rs[:], den[:])
            bcs = small.tile([64, QBW], F32, name="bcs", tag="bcs")
            nc.gpsimd.partition_broadcast(bcs[:], rs[:])
            infos.append((hd, qd, dt, pb, cps, bcs))
        for hd, qd, dt, pb, cps, bcs in infos:
            if pb == 0:
                nc.vector.tensor_tensor(
                    out=cn[dt][0:64, ts(qd, QBW)],
                    in0=cps[0:64, :], in1=bcs[:],
                    op=mybir.AluOpType.mult,
                )
            else:
                tmp = small.tile([64, QBW], MMD, name="tmp", tag="tmp")
                nc.vector.tensor_tensor(
                    out=tmp[:], in0=cps[0:64, :], in1=bcs[:],
                    op=mybir.AluOpType.mult,
                )
                nc.gpsimd.dma_start(cn[dt][64:128, ts(qd, QBW)], tmp[:])
            if hd == HPC - 1:
                for qcx in range(qd * (QBW // P), (qd + 1) * (QBW // P)):
                    queue_oproj(qcx)
        oq.append(nc.scalar)  # scalar queue is free once the exps are done
        while pend_fill:
            pend_fill.pop(0)[2]()

    nc.compile()
    return nc


_NC_CACHE = {}


def _get_nc(S, SKV, bf16=True):
    key = (S, SKV, bf16)
    if key not in _NC_CACHE:
        _NC_CACHE[key] = build_nc(S, SKV, bf16)
    return _NC_CACHE[key]


def _install_ntff_hook():
    try:
        mod = types.ModuleType("antenv.axon_hooks")
        state = {"hook": None}
        mod.set_axon_ntff_profile_hook = lambda h: state.__setitem__("hook", h)
        mod.get_axon_ntff_profile_hook = lambda: state["hook"]
        sys.modules["antenv.axon_hooks"] = mod
        from trn_agent_boot.trn_boot import _ntff_profile_via_ctypes

        mod.set_axon_ntff_profile_hook(
            _ntff_profile_via_ctypes("/opt/axon/libaxon_pjrt.so")
        )
        bass_utils.upload_artifacts = lambda tmpdir: "local://" + tmpdir
        return state["hook"] is not None
    except Exception:
        return False


def run_cores(in_maps, S=2048, SKV=1152, bf16=True, profile=False):
    nc = _get_nc(S, SKV, bf16)
    trace = bool(profile) and _install_ntff_hook()
    res = bass_utils.run_bass_kernel_spmd(
        nc, in_maps, core_ids=list(range(len(in_maps))), trace=trace
    )
    return res


def make_in_maps(q, k, v, mask, Wq, bq, Wk, bk, Wv, Wo, bf16=True):
    B, S, _ = q.shape
    mmd = ml_dtypes.bfloat16 if bf16 else np.float32
    q = np.asarray(q, np.float32)
    k = np.asarray(k, np.float32)
    v = np.asarray(v, np.float32)
    keep = ~np.asarray(mask).reshape(B, S)
    counts = keep.sum(axis=1)
    SKV = max(1024, int(-(-int(counts.max()) // 128)) * 128)
    Wq, Wk, Wv, Wo = (np.asarray(a, np.float32) for a in (Wq, Wk, Wv, Wo))
    bq, bk = np.asarray(bq, np.float32), np.asarray(bk, np.float32)
    in_maps = []
    NKT = SKV // 128
    for b in range(B):
        idx = np.nonzero(keep[b])[0]
        n = len(idx)
        kTc = np.zeros((D, SKV), np.float32)
        kTc[:, :n] = k[b][idx].T
        vTc = np.zeros((D, SKV), np.float32)
        vTc[:, :n] = v[b][idx].T
        mvec = np.zeros(SKV, np.float32)
        mvec[:n] = 1.0
        qTb = np.ascontiguousarray(q[b].T).astype(mmd)
        kTc = kTc.astype(mmd)
        vTc = vTc.astype(mmd)
        for half in range(2):
            hs = slice(DH * half, DH * (half + 1))
            # smalls: col 0..2 bq dt-tiles, 3..5 bk, 6.. mv k-tiles
            sm = np.zeros((128, 6 + NKT), np.float32)
            sm[:, 0:3] = bq[hs].reshape(3, 128).T
            sm[:, 3:6] = bk[hs].reshape(3, 128).T
            sm[:, 6:] = mvec.reshape(NKT, 128).T
            in_maps.append(
                {
                    "qT": qTb,
                    "kT": kTc,
                    "vT": vTc,
                    "wq": np.ascontiguousarray(Wq[:, hs]).astype(mmd),
                    "wk": np.ascontiguousarray(Wk[:, hs]).astype(mmd),
                    "wv": np.ascontiguousarray(Wv[:, hs]).astype(mmd),
                    "wo": np.ascontiguousarray(Wo[hs, :]).astype(mmd),
                    "smalls": sm,
                }
            )
    return in_maps, SKV


def kernel(q, k, v, mask, Wq, bq, Wk, bk, Wv, bv, Wo, bo):
    q = np.asarray(q, np.float32)
    B, S, _ = q.shape
    bf16 = os.environ.get("BASS_PRECISE") != "1"
    in_maps, SKV = make_in_maps(q, k, v, mask, Wq, bq, Wk, bk, Wv, Wo, bf16=bf16)
    res = run_cores(
        in_maps, S=S, SKV=SKV, bf16=bf16,
        profile=os.environ.get("BASS_PROFILE") == "1",
    )
    if os.environ.get("BASS_PROFILE") == "1" and res.exec_time_ns is not None:
        print(f"HW exec time: {res.exec_time_ns} ns")
    cvec = (
        np.asarray(bv, np.float32) @ np.asarray(Wo, np.float32)
        + np.asarray(bo, np.float32)
    )
    out = np.empty((B, S, D), np.float32)
    for b in range(B):
        out[b] = (
            np.asarray(res.results[2 * b]["out"], np.float32)
            + np.asarray(res.results[2 * b + 1]["out"], np.float32)
            + cvec
        )
    return out


# revision 73
# speedup vs baseline: 1.0287x; 1.0078x over previous
"""MultiHeadAttention TRN2 Bass kernel.

Problem: B=4, S=2048, D=768, H=12 heads (DK=64).
Sharding: 8 cores = (batch b in 0..3) x (head-half in 0..1); each core
computes 6 heads of one batch element end-to-end (tensor-parallel over
heads within a batch). Host pre-transposes activations to [D, S] (and
casts to bf16 in the default fast path), slices projection weights per
head-half, and sums the two partial outputs per batch (+ bv@Wo + bo
correction, exact because softmax rows sum to 1).

Key optimization vs the dense formulation: the mask is per-key (same
for every query/head in a batch), so masked keys are removed ENTIRELY
on the host -- k/v are gathered down to the ~50% kept keys and padded
with zeros to SKV (multiple of 128, >= 1024). mv[s]=1 marks real keys,
0 marks padding; it is folded into vh_aug so padded keys contribute
exactly 0 to both the softmax numerator and denominator. This cuts
k/v-proj, scores, exp, and attn@V work by ~44% with bit-identical
semantics to the -inf mask.

On-core math (SKV = padded kept-key count, NKT = SKV/128):
  qh^T[384, S]: lhsT=Wq tile [Din,dout], rhs=q^T tile [Din,s] (+bq)
  kh^T[384, SKV] likewise; vh natural [SKV, 390] via lhsT=v^T, rhs=Wv:
    vh_aug[s, 65j..65j+64] = [mv(s)*vh_head_j(s, :), mv(s)]
  S^T[k, q] = kh_head^T.T @ qh_head^T  (contraction d=64; the two heads
    of a pair land on disjoint PE row quadrants -> they co-execute, and
    share one [128,1024] PSUM tile -> one exp per step)
  P^T = exp(S^T * 0.125)               (ACT, fused scale, no max-sub)
  ctx_aug^T[0:65, q] += vh_aug_j[kc].T @ P^T[kc]  over NKT k-chunks
    rows 0..63 = unnormalized ctx^T, row 64 = softmax denominator
  rs = reciprocal_approx_fast(denom copied to SBUF); bcast on gpsimd;
  cn = ctx^T * rs   (drains deferred so the PE pipeline never waits)
  out[q, 768] = sum_dt cn[dt].T @ Wo tiles  (per 128-q chunk), bf16 out

Scheduling notes (why it runs at ~194us instead of the 394us of the
dense ancestor):
- masked-key compaction cuts scores/exp/attn@V/kv-proj by ~44%
- phase 2 is co-bound: PE ~753ns/step + fillers vs ACT exp ~1.1us/step;
  deferred q-proj and O-proj run as fillers in fixed kc slots, placed
  >=4 steps after the drain that produces their input (in-order PE: a
  stalled filler blocks everything behind it)
- every dma_start rides ONE ~20GB/s hardware ring: big loads are split
  into ~100-150KB pieces issued round-robin over the sync/gpsimd/scalar
  queue sequencers (~600ns issue cost each), in consumption order
- the PE ramps 0.65->1.2->2.4GHz over ~3us of continuous work and any
  idle gap resets it: keep-warm dummy matmuls bridge the final drain
  wait so the tail O-projs run at full clock
- the first two scores+exp are hoisted before v-proj so the scalar
  engine's 108-exp stream starts as early as possible
"""

import os
import sys
import types
from contextlib import ExitStack

import ml_dtypes
import numpy as np

import concourse.bacc as bacc
import concourse.bass as bass
import concourse.mybir as mybir
import concourse.tile as tile
from concourse import bass_utils
from concourse.bass import ts, ds

F32 = mybir.dt.float32
F32R = mybir.dt.float32r
BF16 = mybir.dt.bfloat16

D = 768        # model dim
DH = 384       # per-core head dim (6 heads x 64)
HPC = 6        # heads per core
VW = HPC * 65  # vh_aug free width (390)


def build_nc(S=2048, SKV=1152, bf16=True):
    nc = bacc.Bacc("TRN2", target_bir_lowering=False, debug=False)

    MMD = BF16 if bf16 else F32R    # matmul operand dtype
    NKT = SKV // 128                # 128-wide k-tiles
    assert SKV % 128 == 0 and NKT >= 8
    QBW = min(512, S)               # attention q-block width
    NQB = S // QBW                  # q blocks
    CWQ = min(1024, S)              # q-proj s-chunk width
    # k-proj free-dim chunk: largest 128*d <= 512 with d | NKT
    CWK = next(128 * d for d in (4, 3, 2, 1) if NKT % d == 0)

    qT = nc.dram_tensor("qT", [D, S], MMD, kind="ExternalInput").ap()
    kT = nc.dram_tensor("kT", [D, SKV], MMD, kind="ExternalInput").ap()
    vT = nc.dram_tensor("vT", [D, SKV], MMD, kind="ExternalInput").ap()
    wq = nc.dram_tensor("wq", [D, DH], MMD, kind="ExternalInput").ap()
    wk = nc.dram_tensor("wk", [D, DH], MMD, kind="ExternalInput").ap()
    wv = nc.dram_tensor("wv", [D, DH], MMD, kind="ExternalInput").ap()
    wo = nc.dram_tensor("wo", [DH, D], MMD, kind="ExternalInput").ap()
    # col 0..2 = bq (3 dt-tiles), 3..5 = bk, 6..6+NKT = mv (padding flag)
    smalls = nc.dram_tensor("smalls", [128, 6 + NKT], F32, kind="ExternalInput").ap()
    out = nc.dram_tensor("out", [S, D], BF16, kind="ExternalOutput").ap()

    with tile.TileContext(nc) as tc, ExitStack() as ctx:
        P = 128
        wpool = ctx.enter_context(tc.tile_pool(name="w", bufs=1))
        xin = ctx.enter_context(tc.tile_pool(name="xin", bufs=12))
        persist = ctx.enter_context(tc.tile_pool(name="persist", bufs=1))
        ppool = ctx.enter_context(tc.tile_pool(name="p", bufs=3))
        small = ctx.enter_context(tc.tile_pool(name="small", bufs=2))
        outp = ctx.enter_context(tc.tile_pool(name="outp", bufs=2))
        psA = ctx.enter_context(tc.tile_pool(name="psA", bufs=2, space="PSUM"))
        psB = ctx.enter_context(tc.tile_pool(name="psB", bufs=4, space="PSUM"))

        # Round-robin DMA issue across 4 engine sequencers: each dma_start
        # costs ~600ns of issue time on its engine, so spreading the ~45
        # phase-1 loads over 4 queues (in dependency order: wk+kt first)
        # cuts the serial descriptor-issue head from ~20us to ~4us.
        # The scalar (ACT) queue must drain before exp(0) can dispatch --
        # every DMA issued on it costs ~600ns of sequencer time ahead of the
        # exp stream. So scalar only gets the earliest wave (wk/kt), and is
        # dropped from the rotation afterwards.
        dmaq = [nc.sync, nc.gpsimd, nc.scalar]
        dqi = [0]

        def dq_start(dst, src):
            dmaq[dqi[0] % len(dmaq)].dma_start(dst, src)
            dqi[0] += 1

        # ---- constants / small tensors ----
        wq_sb = [wpool.tile([P, DH], MMD, name=f"wq{c}", tag=f"wq{c}") for c in range(6)]
        wk_sb = [wpool.tile([P, DH], MMD, name=f"wk{c}", tag=f"wk{c}") for c in range(6)]
        wv_sb = [wpool.tile([P, DH], MMD, name=f"wv{c}", tag=f"wv{c}") for c in range(6)]
        wo_sb = [wpool.tile([P, D], MMD, name=f"wo{c}", tag=f"wo{c}") for c in range(3)]
        sm_sb = wpool.tile([128, 6 + NKT], F32, tag="smalls")
        # DMA priority order = compute order: q-proj runs first (so its data
        # loads first), k-proj next (kt fully landed by then -> no mid-kproj
        # DMA stalls that would reset the PE p-state), v/o/deferred-q last.
        # Each dma_start rides a single ~20GB/s hardware ring, so big loads
        # are split into ~128-150KB pieces to spread across the 16 rings,
        # issued in the order compute consumes them (kproj, vproj, qproj).
        # kt/vt pieces are split on the column boundaries the consuming
        # matmul chains use, and issued chunk-0-of-every-tile first, so the
        # first kproj/vproj chains start as early as possible.
        kt = [xin.tile([P, SKV], MMD, name="xin", tag="xin") for c in range(6)]
        for c in range(6):
            dq_start(wk_sb[c][:], wk[ts(c, P), :])
        for sc in range(SKV // CWK):
            for c in range(6):
                dq_start(kt[c][:, ts(sc, CWK)], kT[ts(c, P), ts(sc, CWK)])
        dmaq.pop()  # scalar queue must be free well before the first exp
        dq_start(sm_sb[:], smalls[:, :])
        bq_sb = [sm_sb[:, t : t + 1] for t in range(3)]
        bk_sb = [sm_sb[:, 3 + t : 4 + t] for t in range(3)]
        mv_sb = [sm_sb[:, 6 + st : 7 + st] for st in range(NKT)]
        ones6 = wpool.tile([P, HPC], F32, tag="ones6")
        nc.vector.memset(ones6[:], 1.0)
        qt0 = [xin.tile([P, CWQ], MMD, name="xin", tag="xin") for c in range(6)]
        for c in range(6):
            dq_start(wq_sb[c][:], wq[ts(c, P), :])
            dq_start(qt0[c][:, 0:512], qT[ts(c, P), 0:512])
        vt = [xin.tile([P, SKV], MMD, name="xin", tag="xin") for c in range(6)]
        for c in range(6):
            dq_start(wv_sb[c][:], wv[ts(c, P), :])
        for sc in range(SKV // CWK):
            for c in range(6):
                dq_start(vt[c][:, ts(sc, CWK)], vT[ts(c, P), ts(sc, CWK)])
        for c in range(6):
            dq_start(qt0[c][:, 512:CWQ], qT[ts(c, P), 512:CWQ])
        for c in range(3):
            dq_start(wo_sb[c][:], wo[ts(c, P), :])
        qproj_xt = {0: qt0}
        for sc in range(1, S // CWQ):
            qproj_xt[sc] = [
                xin.tile([P, CWQ], MMD, name="xin", tag="xin") for c in range(6)
            ]
            for c in range(6):
                dq_start(qproj_xt[sc][c][:, 0:512], qT[ts(c, P), ds(sc * CWQ, 512)])
                dq_start(
                    qproj_xt[sc][c][:, 512:CWQ],
                    qT[ts(c, P), ds(sc * CWQ + 512, 512)],
                )

        # ---- persistent activations ----
        khT = [persist.tile([P, SKV], MMD, name=f"khT{t}", tag=f"khT{t}") for t in range(3)]
        qhT = [persist.tile([P, S], MMD, name=f"qhT{t}", tag=f"qhT{t}") for t in range(3)]
        vh = [persist.tile([P, VW], MMD, name=f"vh{st}", tag=f"vh{st}") for st in range(NKT)]
        cn = [persist.tile([P, S], MMD, name=f"cn{t}", tag=f"cn{t}") for t in range(3)]

        # ---- phase 1: k-proj, v-proj, then q-proj of the first 512 cols
        # (matching DMA arrival order); the rest of q-proj is phase-2 filler.
        for sc in range(SKV // CWK):
            for dt in range(3):
                ps = psA.tile([P, CWK], F32, name="psA", tag="psA")
                for c in range(6):
                    nc.tensor.matmul(
                        ps[:],
                        lhsT=wk_sb[c][:, ts(dt, P)],
                        rhs=kt[c][:, ts(sc, CWK)],
                        start=(c == 0),
                        stop=(c == 5),
                    )
                nc.vector.tensor_scalar_add(
                    out=khT[dt][:, ts(sc, CWK)], in0=ps[:],
                    scalar1=bk_sb[dt],
                )

        def vproj_sub(st):
            ps = psB.tile([P, 512], F32, name="psB", tag="psB")
            for c in range(6):
                nc.tensor.matmul(
                    ps[:, :DH],
                    lhsT=vt[c][:, ts(st, P)],
                    rhs=wv_sb[c][:],
                    start=(c == 0),
                    stop=(c == 5),
                )
            vh3 = vh[st].rearrange("p (h c) -> p h c", c=65)
            nc.vector.tensor_scalar_mul(
                out=vh3[:, :, 0:64],
                in0=ps[:, :DH].rearrange("p (h c) -> p h c", c=64),
                scalar1=mv_sb[st],
            )
            nc.vector.tensor_scalar_mul(
                out=vh3[:, :, 64:65],
                in0=ones6[:].rearrange("p (h c) -> p h c", c=1),
                scalar1=mv_sb[st],
            )

        for dt in range(3):
            ps = psA.tile([P, 512], F32, name="psA", tag="psA")
            for c in range(6):
                nc.tensor.matmul(
                    ps[:],
                    lhsT=wq_sb[c][:, ts(dt, P)],
                    rhs=qt0[c][:, ts(0, 512)],
                    start=(c == 0),
                    stop=(c == 5),
                )
            nc.vector.tensor_scalar_add(
                out=qhT[dt][:, ts(0, 512)], in0=ps[:],
                scalar1=bq_sb[dt],
            )
        pend_qproj = [(0, dt, 1) for dt in range(3)] + [
            (sc, dt, u)
            for sc in range(1, S // CWQ)
            for dt in range(3)
            for u in range(CWQ // 512)
        ]

        # ---- phase 2: attention, head-pair steps ----
        # Each step handles BOTH heads of a pair for one k-chunk: the two
        # scores matmuls live in disjoint PE row groups (base partition 0
        # and 64) and share one [128,1024] PSUM tile (head A in cols 0:512,
        # head B in 512:1024) -> one exp per step. Scores run 2 steps ahead
        # of attn@V. Drains and filler work (deferred q-proj, O-proj) go to
        # psB spare slots, never stalling the scores pipeline on psA.
        hq = [(pr, qb) for qb in range(NQB) for pr in range(3)]
        steps = [(pr, qb, kc) for (pr, qb) in hq for kc in range(NKT)]

        ctx_ps = {}
        st_ps = {}

        def scores(pr, qb, kc):
            ps = psA.tile([P, 1024], F32, name="psA", tag="psA")
            for hh in range(2):
                nc.tensor.matmul(
                    ps[:, ts(hh, 512)],
                    lhsT=khT[pr][64 * hh : 64 * hh + 64, ts(kc, P)],
                    rhs=qhT[pr][64 * hh : 64 * hh + 64, ts(qb, QBW)],
                    start=True,
                    stop=True,
                )
            st_ps[(pr, qb, kc)] = ps

        # Hoist the first DEPTH scores ahead of v-proj: they only need
        # khT/qhT, so the scalar engine starts the exp stream while the PE
        # is still working through v-proj.
        scores(*steps[0])
        scores(*steps[1])
        for st in range(NKT):
            vproj_sub(st)

        def attnv(pr, qb, kc, pt):
            for hh in range(2):
                h = 2 * pr + hh
                nc.tensor.matmul(
                    ctx_ps[(h, qb)][0:65, :],
                    lhsT=vh[kc][:, ds(65 * h, 65)],
                    rhs=pt[:, ts(hh, 512)],
                    start=(kc == 0),
                    stop=(kc == NKT - 1),
                )

        def drain(h, qb):
            """Normalize + store ctx for a finished (h, qb)."""
            dt, pb = h // 2, 64 * (h % 2)
            cps = ctx_ps.pop((h, qb))
            den = small.tile([1, QBW], F32, name="den", tag="den")
            nc.vector.tensor_copy(den[:], cps[64:65, :])
            rs = small.tile([1, QBW], F32, name="rs", tag="rs")
            nc.vector.reciprocal_approx_fast(rs[:], den[:])
            bcs = small.tile([64, QBW], F32, name="bcs", tag="bcs")
            nc.gpsimd.partition_broadcast(bcs[:], rs[:])
            if pb == 0:
                nc.vector.tensor_tensor(
                    out=cn[dt][0:64, ts(qb, QBW)],
                    in0=cps[0:64, :],
                    in1=bcs[:],
                    op=mybir.AluOpType.mult,
                )
            else:
                tmp = small.tile([64, QBW], MMD, name="tmp", tag="tmp")
                nc.vector.tensor_tensor(
                    out=tmp[:], in0=cps[0:64, :], in1=bcs[:],
                    op=mybir.AluOpType.mult,
                )
                nc.sync.dma_start(cn[dt][64:128, ts(qb, QBW)], tmp[:])

        oq = [nc.sync, nc.gpsimd]

        # Fillers (deferred q-proj, O-proj) are split into <=3-matmul micro
        # pieces chained via open PSUM accumulation, dispatched ONE per step:
        # a chunky filler between two scores delays the next exp (the phase-2
        # pacer) by its full length, costing ~2us per occurrence. Entries are
        # (allocates_psB, chain_len, fn); allocations only happen at kc >= 4
        # (after the previous group's ctx tiles are released) and early
        # enough that the chain closes within its group.
        pend_fill = []

        def queue_qproj(sc, dt, u):
            box = {}

            def p1():
                box["ps"] = psB.tile([P, 512], F32, name="psB", tag="psB")
                for c in range(3):
                    nc.tensor.matmul(
                        box["ps"][:],
                        lhsT=wq_sb[c][:, ts(dt, P)],
                        rhs=qproj_xt[sc][c][:, ts(u, 512)],
                        start=(c == 0),
                        stop=False,
                    )

            def p2():
                for c in range(3, 6):
                    nc.tensor.matmul(
                        box["ps"][:],
                        lhsT=wq_sb[c][:, ts(dt, P)],
                        rhs=qproj_xt[sc][c][:, ts(u, 512)],
                        start=False,
                        stop=(c == 5),
                    )
                nc.vector.tensor_scalar_add(
                    out=qhT[dt][:, ds(sc * CWQ + u * 512, 512)],
                    in0=box["ps"][:], scalar1=bq_sb[dt],
                )

            pend_fill.append((True, 2, p1))
            pend_fill.append((False, 0, p2))

        def queue_oproj(qc):
            box = {}

            def a1():
                box["ups"] = psB.tile([P, 512], F32, name="psB", tag="psB")
                for dt in range(2):
                    nc.tensor.matmul(
                        box["ups"][:],
                        lhsT=cn[dt][:, ts(qc, P)],
                        rhs=wo_sb[dt][:, ds(0, 512)],
                        start=(dt == 0),
                        stop=False,
                    )

            def a2():
                nc.tensor.matmul(
                    box["ups"][:],
                    lhsT=cn[2][:, ts(qc, P)],
                    rhs=wo_sb[2][:, ds(0, 512)],
                    start=False,
                    stop=True,
                )
                box["ot"] = outp.tile([P, D], MMD, name="ot", tag="ot")
                nc.vector.tensor_copy(box["ot"][:, 0:512], box["ups"][:, 0:512])
                # 64KB pieces: each dma_start rides one ~20GB/s ring, and the
                # last piece's transfer time sits on the kernel's tail
                oq[qc % len(oq)].dma_start(
                    out[ts(qc, P), 0:256], box["ot"][:, 0:256]
                )
                oq[(qc + 1) % len(oq)].dma_start(
                    out[ts(qc, P), 256:512], box["ot"][:, 256:512]
                )

            def b1():
                box["ups2"] = psB.tile([P, 256], F32, name="psB2", tag="psB")
                for dt in range(2):
                    nc.tensor.matmul(
                        box["ups2"][:, 0:256],
                        lhsT=cn[dt][:, ts(qc, P)],
                        rhs=wo_sb[dt][:, ds(512, 256)],
                        start=(dt == 0),
                        stop=False,
                    )

            def b2():
                nc.tensor.matmul(
                    box["ups2"][:, 0:256],
                    lhsT=cn[2][:, ts(qc, P)],
                    rhs=wo_sb[2][:, ds(512, 256)],
                    start=False,
                    stop=True,
                )
                nc.vector.tensor_copy(box["ot"][:, 512:768], box["ups2"][:, 0:256])
                oq[(qc + 2) % len(oq)].dma_start(
                    out[ts(qc, P), 512:768], box["ot"][:, 512:768]
                )

            pend_fill.append((True, 2, a1))
            pend_fill.append((False, 0, a2))
            pend_fill.append((True, 2, b1))
            pend_fill.append((False, 0, b2))

        def queue_vproj(st):
            box = {}

            def v1():
                box["ps"] = psB.tile([P, 512], F32, name="psB", tag="psB")
                for c in range(3):
                    nc.tensor.matmul(
                        box["ps"][:, :DH],
                        lhsT=vt[c][:, ts(st, P)],
                        rhs=wv_sb[c][:],
                        start=(c == 0),
                        stop=False,
                    )

            def v2():
                for c in range(3, 6):
                    nc.tensor.matmul(
                        box["ps"][:, :DH],
                        lhsT=vt[c][:, ts(st, P)],
                        rhs=wv_sb[c][:],
                        start=False,
                        stop=(c == 5),
                    )
                vh3 = vh[st].rearrange("p (h c) -> p h c", c=65)
                nc.vector.tensor_scalar_mul(
                    out=vh3[:, :, 0:64],
                    in0=box["ps"][:, :DH].rearrange("p (h c) -> p h c", c=64),
                    scalar1=mv_sb[st],
                )
                nc.vector.tensor_scalar_mul(
                    out=vh3[:, :, 64:65],
                    in0=ones6[:].rearrange("p (h c) -> p h c", c=1),
                    scalar1=mv_sb[st],
                )

            pend_fill.append((True, 2, v1))
            pend_fill.append((False, 0, v2))

        for e in pend_qproj:
            queue_qproj(*e)

        DEPTH = 2
        pend_drain = []
        for n, (pr, qb, kc) in enumerate(steps):
            if kc == 0:
                for hh in range(2):
                    ctx_ps[(2 * pr + hh, qb)] = psB.tile(
                        [P, QBW], F32, name="psB", tag="psB"
                    )[0:65, :]
            pt = ppool.tile([P, 1024], MMD, name="pt", tag="pt")
            nc.scalar.activation(
                pt[:], st_ps.pop((pr, qb, kc))[:],
                mybir.ActivationFunctionType.Exp, scale=0.125,
            )
            if n + DEPTH < len(steps):
                scores(*steps[n + DEPTH])
            attnv(pr, qb, kc, pt)
            if kc in (0, 1) and pend_drain:
                hd, qd = pend_drain.pop(0)
                drain(hd, qd)
                if hd == HPC - 1:
                    for qcx in range(qd * (QBW // P), (qd + 1) * (QBW // P)):
                        queue_oproj(qcx)
            elif pend_fill:
                na, cl, fn = pend_fill[0]
                if (not na) or (4 <= kc <= NKT - cl):
                    pend_fill.pop(0)
                    fn()
            if kc == NKT - 1:
                pend_drain.extend([(2 * pr, qb), (2 * pr + 1, qb)])
        while pend_fill:
            pend_fill.pop(0)[2]()
        # tail: keep-warm matmuls bridge the drain-wait gap so the final
        # o-proj matmuls run at full PE clock instead of mid p-state
        wps = psA.tile([P, 512], F32, name="psA", tag="psA")
        for _ in range(16):
            nc.tensor.matmul(
                wps[:], lhsT=khT[0][:, 0:128], rhs=khT[0][:, 0:512],
                start=True, stop=True,
            )
        # batch the final drains phase-by-phase so the two DVE chains
        # and the two gpsimd broadcasts interleave instead of serializing
        infos = []
        for hd, qd in pend_drain:
            dt, pb = hd // 2, 64 * (hd % 2)
            cps = ctx_ps.pop((hd, qd))
            den = small.tile([1, QBW], F32, name="den", tag="den")
            nc.vector.tensor_copy(den[:], cps[64:65, :])
            rs = small.tile([1, QBW], F32, name="rs", tag="rs")
            nc.vector.reciprocal_approx_fast(rs[:], den[:])
            bcs = small.tile([64, QBW], F32, name="bcs", tag="bcs")
            nc.gpsimd.partition_broadcast(bcs[:], rs[:])
            infos.append((hd, qd, dt, pb, cps, bcs))
        for hd, qd, dt, pb, cps, bcs in infos:
            if pb == 0:
                nc.vector.tensor_tensor(
                    out=cn[dt][0:64, ts(qd, QBW)],
                    in0=cps[0:64, :], in1=bcs[:],
                    op=mybir.AluOpType.mult,
                )
            else:
                tmp = small.tile([64, QBW], MMD, name="tmp", tag="tmp")
                nc.vector.tensor_tensor(
                    out=tmp[:], in0=cps[0:64, :], in1=bcs[:],
                    op=mybir.AluOpType.mult,
                )
                nc.sync.dma_start(cn[dt][64:128, ts(qd, QBW)], tmp[:])
            if hd == HPC - 1:
                for qcx in range(qd * (QBW // P), (qd + 1) * (QBW // P)):
                    queue_oproj(qcx)
        oq.append(nc.scalar)  # scalar queue is free once the exps are done
        while pend_fill:
            pend_fill.pop(0)[2]()

    nc.compile()
    return nc


_NC_CACHE = {}


def _get_nc(S, SKV, bf16=True):
    key = (S, SKV, bf16)
    if key not in _NC_CACHE:
        _NC_CACHE[key] = build_nc(S, SKV, bf16)
    return _NC_CACHE[key]


def _install_ntff_hook():
    try:
        mod = types.ModuleType("antenv.axon_hooks")
        state = {"hook": None}
        mod.set_axon_ntff_profile_hook = lambda h: state.__setitem__("hook", h)
        mod.get_axon_ntff_profile_hook = lambda: state["hook"]
        sys.modules["antenv.axon_hooks"] = mod
        from trn_agent_boot.trn_boot import _ntff_profile_via_ctypes

        mod.set_axon_ntff_profile_hook(
            _ntff_profile_via_ctypes("/opt/axon/libaxon_pjrt.so")
        )
        bass_utils.upload_artifacts = lambda tmpdir: "local://" + tmpdir
        return state["hook"] is not None
    except Exception:
        return False


def run_cores(in_maps, S=2048, SKV=1152, bf16=True, profile=False):
    nc = _get_nc(S, SKV, bf16)
    trace = bool(profile) and _install_ntff_hook()
    res = bass_utils.run_bass_kernel_spmd(
        nc, in_maps, core_ids=list(range(len(in_maps))), trace=trace
    )
    return res


def make_in_maps(q, k, v, mask, Wq, bq, Wk, bk, Wv, Wo, bf16=True):
    B, S, _ = q.shape
    mmd = ml_dtypes.bfloat16 if bf16 else np.float32
    q = np.asarray(q, np.float32)
    k = np.asarray(k, np.float32)
    v = np.asarray(v, np.float32)
    keep = ~np.asarray(mask).reshape(B, S)
    counts = keep.sum(axis=1)
    SKV = max(1024, int(-(-int(counts.max()) // 128)) * 128)
    Wq, Wk, Wv, Wo = (np.asarray(a, np.float32) for a in (Wq, Wk, Wv, Wo))
    bq, bk = np.asarray(bq, np.float32), np.asarray(bk, np.float32)
    in_maps = []
    NKT = SKV // 128
    for b in range(B):
        idx = np.nonzero(keep[b])[0]
        n = len(idx)
        kTc = np.zeros((D, SKV), np.float32)
        kTc[:, :n] = k[b][idx].T
        vTc = np.zeros((D, SKV), np.float32)
        vTc[:, :n] = v[b][idx].T
        mvec = np.zeros(SKV, np.float32)
        mvec[:n] = 1.0
        qTb = np.ascontiguousarray(q[b].T).astype(mmd)
        kTc = kTc.astype(mmd)
        vTc = vTc.astype(mmd)
        for half in range(2):
            hs = slice(DH * half, DH * (half + 1))
            # smalls: col 0..2 bq dt-tiles, 3..5 bk, 6.. mv k-tiles
            sm = np.zeros((128, 6 + NKT), np.float32)
            sm[:, 0:3] = bq[hs].reshape(3, 128).T
            sm[:, 3:6] = bk[hs].reshape(3, 128).T
            sm[:, 6:] = mvec.reshape(NKT, 128).T
            in_maps.append(
                {
                    "qT": qTb,
                    "kT": kTc,
                    "vT": vTc,
                    "wq": np.ascontiguousarray(Wq[:, hs]).astype(mmd),
                    "wk": np.ascontiguousarray(Wk[:, hs]).astype(mmd),
                    "wv": np.ascontiguousarray(Wv[:, hs]).astype(mmd),
                    "wo": np.ascontiguousarray(Wo[hs, :]).astype(mmd),
                    "smalls": sm,
                }
            )
    return in_maps, SKV


def kernel(q, k, v, mask, Wq, bq, Wk, bk, Wv, bv, Wo, bo):
    q = np.asarray(q, np.float32)
    B, S, _ = q.shape
    bf16 = os.environ.get("BASS_PRECISE") != "1"
    in_maps, SKV = make_in_maps(q, k, v, mask, Wq, bq, Wk, bk, Wv, Wo, bf16=bf16)
    res = run_cores(
        in_maps, S=S, SKV=SKV, bf16=bf16,
        profile=os.environ.get("BASS_PROFILE") == "1",
    )
    if os.environ.get("BASS_PROFILE") == "1" and res.exec_time_ns is not None:
        print(f"HW exec time: {res.exec_time_ns} ns")
    cvec = (
        np.asarray(bv, np.float32) @ np.asarray(Wo, np.float32)
        + np.asarray(bo, np.float32)
    )
    out = np.empty((B, S, D), np.float32)
    for b in range(B):
        out[b] = (
            np.asarray(res.results[2 * b]["out"], np.float32)
            + np.asarray(res.results[2 * b + 1]["out"], np.float32)
            + cvec
        )
    return out
